# revision 53
# baseline (speedup 1.0000x reference)
"""GPT2 block kernel for 8 TRN2 NeuronCores (Bass/Tile, SPMD).

Sharding: the 4096 rows (batch*seq) are split 8 ways -> 512 rows/core
(4 cores per batch element). Core (b, a) owns query tile a (128 rows)
of every key-quarter j of batch b; the host ships each core ONLY its
own 512 rows (natural order). Per quarter each core LayerNorms and
projects K/V for just its own 128 keys, then a 4-core AllGather
([[0..3],[4..7]], HBM bounce buffers) assembles the full 512-key
K^T/V tiles on every core -- no redundant K/V compute. Scores/PV/exp
cover (quarter q) x (own query tiles j >= q), 62.5% of the rectangle,
quarters DESCENDING; the exchange for quarter q-1 overlaps quarter q's
pair loop. The causal mask is a per-core data tensor (multiplicative
0/1 on exp'd scores; block r<a passes, r==a triangular, r>a zero).

K=64 score matmuls for head pairs (2i, 2i+1) sit on partition halves
0:64/64:128 and are issued back-to-back so they run concurrently on
disjoint PE row groups. LN stats/broadcast matmuls use bf16 operands
(fp32 MMs cost 4 cycles/row). LN gamma/beta are folded into the
consuming weights host-side, exp is batched, reciprocals use the fast
DVE approx, proj packs head pairs to a full K=128 contraction. GpSimd
carries only the blocking collective waits during attention.
"""

import sys
from contextlib import ExitStack

import numpy as np

sys.path.insert(0, "/opt/trn_rl_repo")

import concourse.bacc as bacc
import concourse.mybir as mybir
import concourse.tile as tile
from concourse import bass_utils

dt = mybir.dt
F = mybir.ActivationFunctionType
Alu = mybir.AluOpType

D = 1024
S = 2048
Q = 512        # own rows per core
H = 16
HD = 64
INNER = 4096
P = 128
DC = D // P    # 8
IC = INNER // P  # 32
EPS = 1e-5
NQT = 4        # key quarters
KQ = S // NQT  # 512 keys per quarter
NKT = KQ // P  # 4 key tiles of 128 per quarter

_BUILD_CACHE = {}

_DT = {"f32": dt.float32, "f32r": dt.float32r, "bf16": dt.bfloat16, "f16": dt.float16}


def _build(cfg, dbg=False):
    adt = _DT[cfg[0]]   # attention path: qkv/scores/PV operands
    pdt = _DT[cfg[1]]   # proj/fc/mlp path operands
    nc = bacc.Bacc("TRN2", target_bir_lowering=False, debug=False)
    if dbg:
        dbg_xln = nc.dram_tensor("dbg_xln", [NQT, P, DC, KQ], adt,
                                 kind="ExternalOutput")
        dbg_qt = nc.dram_tensor("dbg_qt", [P, DC, Q], adt,
                                kind="ExternalOutput")
        dbg_kt = nc.dram_tensor("dbg_kt", [NQT, P, DC, KQ], adt,
                                kind="ExternalOutput")
        dbg_acc = nc.dram_tensor("dbg_acc", [65, H, Q], dt.float32,
                                 kind="ExternalOutput")
        dbg_et = nc.dram_tensor("dbg_et", [NQT, P, NKT, KQ], adt,
                                kind="ExternalOutput")
        dbg_at2 = nc.dram_tensor("dbg_at2", [P, DC, Q], pdt,
                                 kind="ExternalOutput")
        dbg_h2 = nc.dram_tensor("dbg_h2", [P, DC, Q], dt.float32,
                                kind="ExternalOutput")
        dbg_h2n = nc.dram_tensor("dbg_h2n", [P, DC, Q], pdt,
                                 kind="ExternalOutput")
        dbg_g = nc.dram_tensor("dbg_g", [P, IC // 2, Q], pdt,
                               kind="ExternalOutput")

    hT = nc.dram_tensor("hT", [D, Q], dt.float32, kind="ExternalInput")
    dmask = nc.dram_tensor("dmask", [P, NKT, P], adt, kind="ExternalInput")
    # weights arrive pre-tiled from the host in consumption order
    w_q = nc.dram_tensor("w_q", [DC, P, DC, P], adt, kind="ExternalInput")
    w_k = nc.dram_tensor("w_k", [DC, P, DC, P], adt, kind="ExternalInput")
    w_v = nc.dram_tensor("w_v", [2, P, DC, 512], adt, kind="ExternalInput")
    w_projr = nc.dram_tensor("w_projr", [DC, P, DC, P], pdt, kind="ExternalInput")
    w_fcr = nc.dram_tensor("w_fcr", [IC, P, DC, P], pdt, kind="ExternalInput")
    w_mlpr = nc.dram_tensor("w_mlpr", [2, DC, P, IC // 2, P], pdt, kind="ExternalInput")
    bq = nc.dram_tensor("bq", [P, DC], dt.float32, kind="ExternalInput")
    bk = nc.dram_tensor("bk", [P, DC], dt.float32, kind="ExternalInput")
    bv = nc.dram_tensor("bv", [HD, H], dt.float32, kind="ExternalInput")
    bproj = nc.dram_tensor("bproj", [P, DC], dt.float32, kind="ExternalInput")
    bfc = nc.dram_tensor("bfc", [P, IC], dt.float32, kind="ExternalInput")
    bmlp = nc.dram_tensor("bmlp", [P, DC], dt.float32, kind="ExternalInput")
    outT = nc.dram_tensor("outT", [D, Q], dt.float32, kind="ExternalOutput")

    hT_r = hT.rearrange("(c p) n -> p c n", p=P)

    with tile.TileContext(nc) as tc:
        with (
            tc.tile_pool(name="const", bufs=1) as const,
            tc.tile_pool(name="rows", bufs=2) as rows,
            tc.tile_pool(name="tmp", bufs=2) as tmp,
            tc.tile_pool(name="rowtmp", bufs=2) as rowtmp,
            tc.tile_pool(name="persist", bufs=1) as persist,
        ):
            ones_col = const.tile([P, 1], dt.float32)
            nc.vector.memset(ones_col[:], 1.0)
            ones_row = const.tile([1, P], dt.float32)
            nc.vector.memset(ones_row[:], 1.0)
            # bf16 ones: fp32 matmuls run LOW_HIGH two-pass at 4 cycles/row,
            # so the LN stats / row-broadcast matmuls use bf16 operands
            ones_col_bf = const.tile([P, 1], adt)
            nc.vector.memset(ones_col_bf[:], 1.0)
            ones_row_bf = const.tile([1, P], adt)
            nc.vector.memset(ones_row_bf[:], 1.0)
            eps_t = const.tile([1, 1], dt.float32)
            nc.vector.memset(eps_t[:], EPS)

            def load_pvec(t):
                s = const.tile(list(t.shape), dt.float32, tag=t.name)
                nc.sync.dma_start(s[:], t[:])
                return s

            bq_s, bk_s, bv_s = load_pvec(bq), load_pvec(bk), load_pvec(bv)
            bproj_s, bfc_s, bmlp_s = load_pvec(bproj), load_pvec(bfc), load_pvec(bmlp)

            h2 = persist.tile([P, DC, Q], dt.float32, tag="h2")
            hq_sb = persist.tile([P, DC, Q], dt.float32, tag="hq")

            # LN stats for a [P, DC, KQ] fp32 block resident in SBUF.
            # The block is first cast to bf16 into `xln` (scalar ACT copies);
            # stats read the raw-bf16 cast (partition-sums via bf16 PE
            # ones-matmuls at 1 cycle/row — fp32 MMs would cost 4x), and the
            # apply pass later overwrites xln in place. sq on GpSimd, row
            # math on DVE, sqrt on scalar, reciprocal via fast DVE approx.
            # Per-element bf16 rounding averages out over the 1024-wide sums.
            def ln_stats(get_chunk, xln, lnps, tag, rowtag="lnrow", W=KQ):
                pss = lnps.tile([1, W], dt.float32, tag=rowtag)
                psq = lnps.tile([1, W], dt.float32, tag=rowtag)
                for c in range(DC):
                    nc.scalar.activation(xln[:, c, :], get_chunk(c), F.Copy)
                    sq = tmp.tile([P, W], adt, tag="sq")
                    nc.vector.tensor_tensor(sq[:], xln[:, c, :], xln[:, c, :],
                                            Alu.mult)
                    nc.tensor.matmul(pss[:], ones_col_bf[:], xln[:, c, :],
                                     start=(c == 0), stop=(c == DC - 1))
                    nc.tensor.matmul(psq[:], ones_col_bf[:], sq[:],
                                     start=(c == 0), stop=(c == DC - 1))
                mean = rows.tile([1, W], dt.float32, tag="mean")
                nc.vector.tensor_scalar_mul(mean[:], pss[:], 1.0 / D)
                msq = rowtmp.tile([1, W], dt.float32, tag="lnrow")
                nc.vector.tensor_tensor(msq[:], mean[:], mean[:], Alu.mult)
                var = rowtmp.tile([1, W], dt.float32, tag="lnrow")
                nc.vector.scalar_tensor_tensor(
                    var[:], psq[:], 1.0 / D, msq[:], Alu.mult, Alu.subtract
                )
                std = rowtmp.tile([1, W], dt.float32, tag="lnrow")
                nc.scalar.activation(std[:], var[:], F.Sqrt, bias=eps_t[:])
                rstd = rows.tile([1, W], dt.float32, tag="rstd")
                nc.vector.reciprocal_approx_fast(rstd[:], std[:])
                mean_bf = rows.tile([1, W], adt, tag="meanbf")
                nc.vector.tensor_copy(mean_bf[:], mean[:])
                rstd_bf = rows.tile([1, W], adt, tag="rstdbf")
                nc.vector.tensor_copy(rstd_bf[:], rstd[:])
                return mean_bf, rstd_bf

            # apply pass: xln = (x - mean) * rstd (gamma/beta folded into
            # the consuming weights host-side), written as `odt`.
            def ln_apply_start(mean, rstd, bbpool, bbtag):
                W = mean.shape[-1]
                mb = bbpool.tile([P, W], dt.float32, tag=bbtag)
                rb = bbpool.tile([P, W], dt.float32, tag=bbtag)
                nc.tensor.matmul(mb[:], ones_row_bf[:], mean[:],
                                 start=True, stop=True)
                nc.tensor.matmul(rb[:], ones_row_bf[:], rstd[:],
                                 start=True, stop=True)
                return mb, rb

            def ln_apply_chunk(get_chunk, mb, rb, out, c, gp=False):
                eng = nc.gpsimd if gp else nc.vector
                xc = get_chunk(c)
                t1 = tmp.tile([P, mb.shape[-1]], adt, tag="lnt1")
                eng.tensor_tensor(t1[:], xc, mb[:], Alu.subtract)
                eng.tensor_tensor(out[:, c, :], t1[:], rb[:], Alu.mult)

            def ln_apply(get_chunk, mean, rstd, out, bbpool, bbtag,
                         gp_chunks=0):
                mb, rb = ln_apply_start(mean, rstd, bbpool, bbtag)
                for c in range(DC):
                    ln_apply_chunk(get_chunk, mb, rb, out, c,
                                   gp=(c >= DC - gp_chunks))

            with ExitStack() as attn_st:
                pools = {
                    "attnsc": dict(bufs=1), "kvsb": dict(bufs=2),
                    "xlnp": dict(bufs=4), "h2bfp": dict(bufs=1),
                    "wkv": dict(bufs=3), "wvp": dict(bufs=2),
                    "kownp": dict(bufs=4), "vonp": dict(bufs=2),
                    "etp": dict(bufs=2), "dramp": dict(bufs=4, space="DRAM"),
                    "scp": dict(bufs=3, space="PSUM"),
                    "fillp": dict(bufs=1, space="PSUM"),
                    "pvps": dict(bufs=2, space="PSUM"),
                    "lnps": dict(bufs=2, space="PSUM"),
                }
                pl = {
                    name: attn_st.enter_context(tc.tile_pool(name=name, **kw))
                    for name, kw in pools.items()
                }
                attnsc, kvsb, xlnp, h2bfp = (pl["attnsc"], pl["kvsb"],
                                             pl["xlnp"], pl["h2bfp"])
                wkv, wvp, kownp, vonp = (pl["wkv"], pl["wvp"],
                                         pl["kownp"], pl["vonp"])
                etp, dramp, scp = pl["etp"], pl["dramp"], pl["scp"]
                fillp, pvps, lnps = pl["fillp"], pl["pvps"], pl["lnps"]
                del pl, pools
                qt = attnsc.tile([P, DC, Q], adt, tag="qt")
                attn_acc = attnsc.tile([65, H, Q], dt.float32, tag="attn_acc")
                dmask_s = const.tile([P, NKT, P], adt, tag="dmask")
                nc.sync.dma_start(dmask_s[:], dmask[:])

                # per-quarter K^T / V SBUF tiles (double-buffered: quarter q
                # consumed while q-1 arrives from the all-gather)
                kt_t, v_t = {}, {}

                def new_kv_tiles(qq):
                    kt_sb = kvsb.tile([P, DC, KQ], adt, tag="kt")
                    v_sb = kvsb.tile([P, NKT, H * 65], adt, tag="v")
                    vview = v_sb[:].rearrange("p k (h x) -> p k h x", x=65)
                    nc.vector.tensor_copy(
                        vview[:, :, :, 64:65],
                        ones_col[:].to_broadcast([P, NKT, H, 1]),
                    )
                    kt_t[qq], v_t[qq] = kt_sb, v_sb
                    return kt_sb, v_sb

                def hchunk(qq):
                    return lambda c: hq_sb[:, c, qq * P:(qq + 1) * P]

                def xchunk(xln):
                    return lambda c: xln[:, c, :]

                # own-block projections (128 own keys per quarter); K gets
                # its bias before the exchange, V's rides in emit_norm.
                # Weight tiles are hoisted: one w_k[p]/w_q[p] load serves all
                # four quarters (p-outer loops).
                def g_kown(xln, wk_t, kown, p):
                    psk = fillp.tile([P, P], dt.float32, tag="mm")
                    for c in range(DC):
                        nc.tensor.matmul(
                            psk[:], wk_t[:, c, :], xln[:, c, :],
                            start=(c == 0), stop=(c == DC - 1),
                        )
                    nc.vector.tensor_scalar_add(
                        kown[:, p, :], psk[:], bk_s[:, p:p + 1]
                    )

                def g_vown(xln, wv_t, von, vs):
                    psv = fillp.tile([P, KQ], dt.float32, tag="mm")
                    for c in range(DC):
                        nc.tensor.matmul(
                            psv[:], xln[:, c, :], wv_t[:, c, :],
                            start=(c == 0), stop=(c == DC - 1),
                        )
                    nc.vector.tensor_copy(
                        von[:, vs * 512:(vs + 1) * 512], psv[:]
                    )

                def g_q(xln, wq_t, qq, p):
                    psq_ = fillp.tile([P, P], dt.float32, tag="mm")
                    for c in range(DC):
                        nc.tensor.matmul(
                            psq_[:], wq_t[:, c, :], xln[:, c, :],
                            start=(c == 0), stop=(c == DC - 1),
                        )
                    nc.vector.tensor_scalar_add(
                        qt[:, p, qq * P:(qq + 1) * P], psq_[:],
                        bq_s[:, p:p + 1],
                    )

                snd_t, rcv_t = {}, {}

                def kv_send(qq, kown, von):
                    # own 128-key K/V block -> HBM staging -> 4-core
                    # AllGather (gpsimd queue; the rings run in background)
                    snd = dramp.tile([P, 2048], adt, tag="snd")
                    rcv = dramp.tile([4, P, 2048], adt, tag="rcv")
                    snd_t[qq], rcv_t[qq] = snd, rcv
                    nc.sync.dma_start(
                        snd[:, 0:1024], kown[:].rearrange("p c n -> p (c n)")
                    )
                    nc.sync.dma_start(snd[:, 1024:2048], von[:])
                    nc.gpsimd.collective_compute(
                        "AllGather", Alu.bypass,
                        replica_groups=[[0, 1, 2, 3], [4, 5, 6, 7]],
                        ins=[snd[:].opt()], outs=[rcv[:].opt()],
                    )

                def kv_receive(qq):
                    # 2 batched sync-queue triggers; the CC-completion wait
                    # head-blocks the sync queue, so receives are issued only
                    # when nothing urgent sits behind them
                    rcv = rcv_t.pop(qq)
                    kt_sb, v_sb = new_kv_tiles(qq)
                    for r in range(4):
                        nc.sync.dma_start(
                            kt_sb[:, :, r * P:(r + 1) * P],
                            rcv[r, :, 0:1024].rearrange(
                                "p (c n) -> p c n", n=P),
                        )
                        nc.sync.dma_start(
                            v_sb[:, r, :].rearrange(
                                "p (h x) -> p h x", x=65)[:, :, 0:64],
                            rcv[r, :, 1024:2048].rearrange(
                                "p (h x) -> p h x", x=64),
                        )

                def run_all(gen):
                    for _ in gen:
                        pass

                attnT2 = attnsc.tile([P, DC, Q], pdt, tag="attnT2")

                def emit_norm(h):
                    # per-head softmax normalization, interleaved right
                    # after head h's last (q=0) PV accumulation
                    srow = rowtmp.tile([1, Q], dt.float32, tag="srow")
                    nc.vector.tensor_copy(srow[:], attn_acc[64:65, h, :])
                    rrow = rowtmp.tile([1, Q], dt.float32, tag="rrow")
                    nc.vector.reciprocal_approx_fast(rrow[:], srow[:])
                    rrow_bf = rowtmp.tile([1, Q], adt, tag="rrowbf")
                    nc.vector.tensor_copy(rrow_bf[:], rrow[:])
                    bc = lnps.tile([P, Q], dt.float32, tag="lnrow")
                    nc.tensor.matmul(
                        bc[0:64, :], ones_row_bf[0:1, 0:64], rrow_bf[:],
                        start=True, stop=True,
                    )
                    t1 = tmp.tile([HD, Q], adt, tag="anorm")
                    nc.vector.tensor_tensor(
                        t1[:], attn_acc[0:64, h, :], bc[0:64, :], Alu.mult
                    )
                    off = 64 * (h // 8)
                    nc.vector.tensor_scalar_add(
                        attnT2[off:off + 64, h % 8, :], t1[:], bv_s[:, h:h + 1]
                    )

                def emit_proj(mo, h2bf):
                    wp_t = wkv.tile([P, DC, P], pdt, tag="wkq")
                    nc.sync.dma_start(wp_t[:], w_projr[mo])
                    psp = scp.tile([P, KQ], dt.float32, tag="sc")
                    for c in range(DC):
                        nc.tensor.matmul(
                            psp[:], wp_t[:, c, :], attnT2[:, c, :],
                            start=(c == 0), stop=(c == DC - 1),
                        )
                    nc.vector.scalar_tensor_tensor(
                        h2[:, mo, :], psp[:], bproj_s[:, mo:mo + 1],
                        hq_sb[:, mo, :], Alu.add, Alu.add,
                    )
                    # bf16 mirror feeds the LN2 stats matmuls + apply
                    nc.scalar.activation(h2bf[:, mo, :], h2[:, mo, :], F.Copy)

                et_t = {}

                def emit_scores_pair(kt_sb, q, i, NQ, qsl, pull):
                    # Heads 2i / 2i+1 live on partition halves 0:64 / 64:128
                    # of chunk i. Their K=64 score MMs auto-derive
                    # tile_position (0,0) / (64,0); issued back-to-back they
                    # run CONCURRENTLY on disjoint PE row groups (~2x).
                    he, ho = 2 * i, 2 * i + 1
                    ete = etp.tile([P, NKT, KQ], adt, tag="et")
                    eto = etp.tile([P, NKT, KQ], adt, tag="et")
                    et_t[he], et_t[ho] = ete, eto
                    if NQ <= 256:
                        # [P, 2, NQ] fits one PSUM bank: pair the score MMs
                        # and halve the exp-ACT count
                        for half in range(2):
                            pse = scp.tile([P, 2, 256], dt.float32, tag="sc")
                            pso = scp.tile([P, 2, 256], dt.float32, tag="sc")
                            for k2 in range(2):
                                kt = half * 2 + k2
                                nc.tensor.matmul(
                                    pse[:, k2, 0:NQ],
                                    kt_sb[0:64, i, kt * P:(kt + 1) * P],
                                    qt[0:64, i, qsl],
                                    start=True, stop=True,
                                )
                                nc.tensor.matmul(
                                    pso[:, k2, 0:NQ],
                                    kt_sb[64:128, i, kt * P:(kt + 1) * P],
                                    qt[64:128, i, qsl],
                                    start=True, stop=True,
                                )
                                pull()
                            for et, psc in ((ete, pse), (eto, pso)):
                                nc.scalar.activation(
                                    et[:, half * 2:half * 2 + 2, 0:NQ],
                                    psc[:, :, 0:NQ], F.Exp, scale=0.125,
                                )
                                nc.vector.tensor_tensor(
                                    et[:, half * 2:half * 2 + 2, 0:P],
                                    et[:, half * 2:half * 2 + 2, 0:P],
                                    dmask_s[:, half * 2:half * 2 + 2, :],
                                    Alu.mult,
                                )
                                pull()
                            pull()
                        return
                    for kt in range(NKT):
                        pse = scp.tile([P, KQ], dt.float32, tag="sc")
                        pso = scp.tile([P, KQ], dt.float32, tag="sc")
                        nc.tensor.matmul(
                            pse[:, 0:NQ],
                            kt_sb[0:64, i, kt * P:(kt + 1) * P],
                            qt[0:64, i, qsl],
                            start=True, stop=True,
                        )
                        nc.tensor.matmul(
                            pso[:, 0:NQ],
                            kt_sb[64:128, i, kt * P:(kt + 1) * P],
                            qt[64:128, i, qsl],
                            start=True, stop=True,
                        )
                        pull()
                        nc.scalar.activation(
                            ete[:, kt, 0:NQ], pse[:, 0:NQ], F.Exp, scale=0.125,
                        )
                        nc.scalar.activation(
                            eto[:, kt, 0:NQ], pso[:, 0:NQ], F.Exp, scale=0.125,
                        )
                        if kt % 2 == 1:
                            # multiplicative causal mask on the diagonal
                            # query tile (first 128 columns of the window)
                            nc.vector.tensor_tensor(
                                ete[:, kt - 1:kt + 1, 0:P],
                                ete[:, kt - 1:kt + 1, 0:P],
                                dmask_s[:, kt - 1:kt + 1, :], Alu.mult,
                            )
                            nc.vector.tensor_tensor(
                                eto[:, kt - 1:kt + 1, 0:P],
                                eto[:, kt - 1:kt + 1, 0:P],
                                dmask_s[:, kt - 1:kt + 1, :], Alu.mult,
                            )
                        pull()

                def emit_pv(v_sb, q, h, NQ, qsl, pull=None):
                    et = et_t.pop(h)
                    pa = pvps.tile([65, KQ], dt.float32, tag="pv")
                    for kt in range(NKT):
                        nc.tensor.matmul(
                            pa[:, qsl], v_sb[:, kt, h * 65:h * 65 + 65],
                            et[:, kt, 0:NQ],
                            start=(kt == 0), stop=(kt == NKT - 1),
                        )
                        if pull is not None:
                            pull()
                    nc.vector.tensor_copy(
                        attn_acc[:, h, q * P:(q + 1) * P],
                        pa[:, q * P:(q + 1) * P],
                    )
                    if q < NQT - 1:
                        nc.vector.tensor_tensor(
                            attn_acc[:, h, (q + 1) * P:],
                            attn_acc[:, h, (q + 1) * P:],
                            pa[:, (q + 1) * P:], Alu.add,
                        )
                    if q == 0:
                        emit_norm(h)

                # prologue: the whole LN1 + own-key K/V pipeline for ALL
                # quarters runs up front (it depends only on hT), feeding the
                # ring back-to-back: CC(3) is partially exposed, CC(2..0)
                # pipeline under the attention quarters. ONE batched hT DMA
                # and p-outer weight hoisting keep the serial sync-engine
                # trigger queue short so the sends fire early.
                nc.sync.dma_start(hq_sb[:], hT_r[:])
                wv0 = wvp.tile([P, DC, 512], adt, tag="wv")
                nc.sync.dma_start(wv0[:], w_v[0])
                wv1 = wvp.tile([P, DC, 512], adt, tag="wv")
                nc.sync.dma_start(wv1[:], w_v[1])

                def prep_ln(qq):
                    xln_n = xlnp.tile([P, DC, P], adt, tag="xln")
                    mean_n, rstd_n = ln_stats(hchunk(qq), xln_n, pvps,
                                              str(qq), rowtag="pv", W=P)
                    mb_n, rb_n = ln_apply_start(mean_n, rstd_n, lnps, "lnrow")
                    for c in range(DC):
                        ln_apply_chunk(xchunk(xln_n), mb_n, rb_n, xln_n, c)
                    return xln_n

                xlns, kowns = {}, {}
                for qq in (3, 2, 1, 0):
                    xlns[qq] = prep_ln(qq)
                    kowns[qq] = kownp.tile([P, DC, P], adt, tag="kown",
                                           name=f"kown{qq}")
                for p in range(DC):
                    wk_t = wkv.tile([P, DC, P], adt, tag="wkq")
                    nc.sync.dma_start(wk_t[:], w_k[p])
                    for qq in (3, 2, 1, 0):
                        g_kown(xlns[qq], wk_t, kowns[qq], p)
                for qq in (3, 2, 1, 0):
                    von = vonp.tile([P, 1024], adt, tag="von")
                    g_vown(xlns[qq], wv0, von, 0)
                    g_vown(xlns[qq], wv1, von, 1)
                    kv_send(qq, kowns[qq], von)
                for p in range(DC):
                    wq_t = wkv.tile([P, DC, P], adt, tag="wkq")
                    nc.sync.dma_start(wq_t[:], w_q[p])
                    for qq in (3, 2, 1, 0):
                        g_q(xlns[qq], wq_t, qq, p)
                kv_receive(3)
                kv_receive(2)

                def pull():
                    pass

                for q in range(NQT - 1, -1, -1):
                    kt_sb, v_sb = kt_t.pop(q), v_t.pop(q)
                    if q >= 2:
                        # receive for quarter q-2: its kvsb slot's previous
                        # readers (quarter q's pair loop) are all issued, and
                        # its CC-wait head-blocks the sync queue only while
                        # nothing urgent sits behind it
                        kv_receive(q - 2)
                    NQ = (NQT - q) * P
                    qsl = slice(q * P, Q)
                    # pv(2i-1) precedes pair i's allocation so the 2-buf et
                    # rotation sees every consumer before slot reuse
                    for i in range(H // 2):
                        if i > 0:
                            emit_pv(v_sb, q, 2 * i - 1, NQ, qsl)
                        emit_scores_pair(kt_sb, q, i, NQ, qsl, pull)
                        emit_pv(v_sb, q, 2 * i, NQ, qsl)
                    emit_pv(v_sb, q, H - 1, NQ, qsl)

                if dbg:
                    nc.sync.dma_start(dbg_qt[:], qt[:])
                    nc.sync.dma_start(dbg_acc[:], attn_acc[:])

                h2bf = h2bfp.tile([P, DC, Q], adt, tag="h2bf")
                for mo in range(DC):
                    emit_proj(mo, h2bf)

                # LN2 stats (h2 complete after proj), bf16 operands
                pss2 = pvps.tile([1, KQ], dt.float32, tag="pv")
                psq2 = pvps.tile([1, KQ], dt.float32, tag="pv")
                for mo in range(DC):
                    sq2 = tmp.tile([P, KQ], adt, tag="sq")
                    nc.gpsimd.tensor_tensor(
                        sq2[:], h2bf[:, mo, :], h2bf[:, mo, :], Alu.mult
                    )
                    nc.tensor.matmul(pss2[:], ones_col_bf[:], h2bf[:, mo, :],
                                     start=(mo == 0), stop=(mo == DC - 1))
                    nc.tensor.matmul(psq2[:], ones_col_bf[:], sq2[:],
                                     start=(mo == 0), stop=(mo == DC - 1))
                mean2 = rows.tile([1, KQ], dt.float32, tag="mean")
                nc.vector.tensor_scalar_mul(mean2[:], pss2[:], 1.0 / D)
                msq2 = rowtmp.tile([1, KQ], dt.float32, tag="lnrow")
                nc.vector.tensor_tensor(msq2[:], mean2[:], mean2[:], Alu.mult)
                var2 = rowtmp.tile([1, KQ], dt.float32, tag="lnrow")
                nc.vector.scalar_tensor_tensor(
                    var2[:], psq2[:], 1.0 / D, msq2[:], Alu.mult, Alu.subtract
                )
                std2 = rowtmp.tile([1, KQ], dt.float32, tag="lnrow")
                nc.scalar.activation(std2[:], var2[:], F.Sqrt, bias=eps_t[:])
                rstd2 = rows.tile([1, KQ], dt.float32, tag="rstd")
                nc.vector.reciprocal_approx_fast(rstd2[:], std2[:])
                mean2_bf = rows.tile([1, KQ], adt, tag="meanbf")
                nc.vector.tensor_copy(mean2_bf[:], mean2[:])
                rstd2_bf = rows.tile([1, KQ], adt, tag="rstdbf")
                nc.vector.tensor_copy(rstd2_bf[:], rstd2[:])
                if dbg:
                    nc.sync.dma_start(dbg_at2[:], attnT2[:])

            # ---- LN2 / fc+gelu / mlp + residual ----
            with (
                tc.tile_pool(name="mlpsc", bufs=1) as mlpsc,
                tc.tile_pool(name="wfcs", bufs=4) as wfcs,
                tc.tile_pool(name="wmlps", bufs=4) as wmlps,
                tc.tile_pool(name="psfc", bufs=2, space="PSUM") as psfc,
                tc.tile_pool(name="psm", bufs=2, space="PSUM") as psm,
                tc.tile_pool(name="lnps2", bufs=2, space="PSUM") as lnps2,
            ):
                h2c = lambda c: h2[:, c, :]
                h2n = mlpsc.tile([P, DC, Q], pdt, tag="h2n")
                ln_apply(h2c, mean2_bf, rstd2_bf, h2n, lnps2, "lnbb")
                if dbg:
                    nc.sync.dma_start(dbg_h2[:], h2[:])
                    nc.sync.dma_start(dbg_h2n[:], h2n[:])
                y2 = mlpsc.tile([P, DC, Q], dt.float32, tag="y2")
                g_half = mlpsc.tile([P, IC // 2, Q], pdt, tag="g")
                for ih in range(2):
                    for m in range(IC // 2):
                        mg = ih * (IC // 2) + m
                        wfc_t = wfcs.tile([P, DC, P], pdt, tag="wfc")
                        nc.sync.dma_start(wfc_t[:], w_fcr[mg])
                        psf = psfc.tile([P, Q], dt.float32, tag="fc")
                        for c in range(DC):
                            nc.tensor.matmul(
                                psf[:], wfc_t[:, c, :], h2n[:, c, :],
                                start=(c == 0), stop=(c == DC - 1),
                            )
                        nc.scalar.activation(
                            g_half[:, m, :], psf[:], F.Gelu,
                            bias=bfc_s[:, mg:mg + 1],
                        )
                    if dbg and ih == 0:
                        nc.sync.dma_start(dbg_g[:], g_half[:])
                    for mo in range(DC):
                        wm_t = wmlps.tile([P, IC // 2, P], pdt, tag="wmlp")
                        nc.sync.dma_start(wm_t[:], w_mlpr[ih, mo])
                        psm_ = psm.tile([P, Q], dt.float32, tag="mm2")
                        for c in range(IC // 2):
                            nc.tensor.matmul(
                                psm_[:], wm_t[:, c, :], g_half[:, c, :],
                                start=(c == 0), stop=(c == IC // 2 - 1),
                            )
                        if ih == 0:
                            nc.vector.tensor_copy(y2[:, mo, :], psm_[:])
                        else:
                            ot = tmp.tile([P, Q], dt.float32, tag="anorm")
                            nc.vector.tensor_tensor(
                                ot[:], y2[:, mo, :], psm_[:], Alu.add
                            )
                            nc.vector.scalar_tensor_tensor(
                                ot[:], ot[:], bmlp_s[:, mo:mo + 1],
                                h2[:, mo, :], Alu.add, Alu.add,
                            )
                            nc.sync.dma_start(
                                outT.rearrange("(c p) n -> p c n", p=P)[:, mo, :],
                                ot[:],
                            )

    nc.compile()
    return nc


def _get_nc(cfg):
    if cfg not in _BUILD_CACHE:
        _BUILD_CACHE[cfg] = _build(cfg)
    return _BUILD_CACHE[cfg]


def _np_dt(name):
    if name == "bf16":
        import ml_dtypes
        return ml_dtypes.bfloat16
    if name == "f16":
        return np.float16
    return np.float32


def _prep_in_maps(inputs, cfg):
    adt_np, pdt_np = _np_dt(cfg[0]), _np_dt(cfg[1])
    h = np.asarray(inputs["hidden_states"], dtype=np.float32)
    w_qkv = np.asarray(inputs["w_qkv"], np.float32)
    b_qkv = np.asarray(inputs["b_qkv"], np.float32)
    g1 = np.asarray(inputs["g1"], np.float32)
    be1 = np.asarray(inputs["be1"], np.float32)
    g2 = np.asarray(inputs["g2"], np.float32)
    be2 = np.asarray(inputs["be2"], np.float32)

    # fold LN1 gamma/beta into the qkv weights and biases
    w_qkv_f = w_qkv * g1[:, None]
    b_qkv_f = b_qkv + be1 @ w_qkv
    w_fc = np.asarray(inputs["w_fc"], np.float32)
    b_fc = np.asarray(inputs["b_fc"], np.float32)
    w_fc_f = w_fc * g2[:, None]
    b_fc_f = b_fc + be2 @ w_fc

    def chunk_w(w, p=P):  # [Din, N] -> [p, Din//p, N]
        return np.ascontiguousarray(w.reshape(-1, p, w.shape[1]).transpose(1, 0, 2))

    def pvec(v, p=P):  # [n*p] -> [p, n]
        return np.ascontiguousarray(v.reshape(-1, p).T)

    def mslice(a, nsl):  # [p, c, n] -> [n//nsl, p, c, nsl]
        p, c, n = a.shape
        return np.ascontiguousarray(
            a.reshape(p, c, n // nsl, nsl).transpose(2, 0, 1, 3)
        )

    wq = mslice(chunk_w(w_qkv_f[:, 0:D]), P)
    wk = mslice(chunk_w(w_qkv_f[:, D:2 * D]), P)
    wv = mslice(chunk_w(w_qkv_f[:, 2 * D:3 * D]), 512)

    # proj with head pairs (c, c+8) stacked on the 128 contraction rows
    w_proj = np.asarray(inputs["w_proj"], np.float32)
    w2 = w_proj.reshape(H, HD, D)
    wp_t = np.concatenate([w2[0:8], w2[8:16]], axis=1)  # [8, 128, 1024]
    wp = np.ascontiguousarray(
        wp_t.reshape(DC, P, DC, P).transpose(2, 1, 0, 3)
    )

    wfc = mslice(chunk_w(w_fc_f), P)
    wm = chunk_w(np.asarray(inputs["w_mlp"], np.float32))  # [128, 32, 1024]
    wmlp = np.ascontiguousarray(
        wm.reshape(P, 2, IC // 2, DC, P).transpose(1, 3, 0, 2, 4)
    )

    shared = {
        "w_q": wq.astype(adt_np), "w_k": wk.astype(adt_np),
        "w_v": wv.astype(adt_np), "w_projr": wp.astype(pdt_np),
        "w_fcr": wfc.astype(pdt_np), "w_mlpr": wmlp.astype(pdt_np),
        "bq": pvec(b_qkv_f[0:D]),
        "bk": pvec(b_qkv_f[D:2 * D]),
        "bv": pvec(b_qkv_f[2 * D:3 * D], p=HD),
        "bproj": pvec(np.asarray(inputs["b_proj"], np.float32)),
        "bfc": pvec(b_fc_f),
        "bmlp": pvec(np.asarray(inputs["b_mlp"], np.float32)),
    }
    in_maps = []
    for core in range(8):
        b, a = core // 4, core % 4
        # own rows only (query tile a of every quarter), natural order;
        # K/V for the full quarter arrive via the 4-core all-gather
        rows = np.concatenate(
            [np.arange(KQ * j + P * a, KQ * j + P * a + P) for j in range(NQT)]
        )
        # multiplicative 0/1 mask on exp'd scores for the diagonal query
        # tile: key block r vs own block a — earlier blocks pass, own is
        # triangular, later blocks are fully masked.
        pp = np.arange(P)
        dm = np.zeros((P, NKT, P), np.float32)
        for r in range(NKT):
            if r < a:
                dm[:, r, :] = 1.0
            elif r == a:
                dm[:, r, :] = (pp[:, None] <= pp[None, :]).astype(np.float32)
        in_maps.append(
            dict(
                shared,
                hT=np.ascontiguousarray(h[b][rows].T),
                dmask=dm.astype(adt_np),
            )
        )
    return in_maps


def _stitch(results):
    out = np.empty((2, S, D), dtype=np.float32)
    for core in range(8):
        b, a = core // 4, core % 4
        r = results[core]["outT"].T  # [512, D]: cols j*128+p -> row 512j+128a+p
        for j in range(NQT):
            out[b, j * KQ + P * a: j * KQ + P * a + P] = r[j * P:(j + 1) * P]
    return out


def run(inputs, cfg=("bf16", "bf16"), trace=False, trace_cores=None):
    nc = _get_nc(cfg)
    in_maps = _prep_in_maps(inputs, cfg)
    res = bass_utils.run_bass_kernel_spmd(
        nc, in_maps, core_ids=list(range(8)), trace=trace, trace_cores=trace_cores
    )
    return _stitch(res.results), res


def kernel(**inputs) -> np.ndarray:
    out, _ = run(inputs, cfg=("bf16", "bf16"))
    return out



# revision 58
# speedup vs baseline: 1.1735x; 1.1735x over previous
"""GPT2 block kernel for 8 TRN2 NeuronCores (Bass/Tile, SPMD).

Sharding: the 4096 rows (batch*seq) are split 8 ways -> 512 rows/core
(4 cores per batch element). Core (b, a) owns query blocks {4j + a}
(128 rows each, one per key-quarter j) of batch b. Each core
redundantly computes K,V for its batch, but scores/PV/exp only for
(quarter q) x (own query tiles j >= q) -- 62.5% of the full rectangle.
Quarters are processed in DESCENDING order so query tile j (produced
from quarter j's LayerNorm output) exists before quarters q < j consume
it. Zero collectives.

Per-core key permutation (host side): within each quarter, the core's
own 128 rows are moved to the last 128 key positions, so the Q-proj
input is always xln[:, :, 384:512] -- uniform addresses across cores;
the causal mask becomes a per-core data tensor (multiplicative 0/1 on
the exp'd scores, applied by the otherwise-idle GpSimd engine).

LN gamma/beta are folded into the consuming weights host-side, exp is
batched into [128, 2, N] activations, reciprocals use the fast approx
DVE op, and proj packs head pairs to a full K=128 contraction.
"""

import numpy as np
import sys

sys.path.insert(0, "/opt/trn_rl_repo")

import concourse.bacc as bacc
import concourse.mybir as mybir
import concourse.tile as tile
from concourse import bass_utils

dt = mybir.dt
F = mybir.ActivationFunctionType
Alu = mybir.AluOpType

D = 1024
S = 2048
Q = 512        # own rows per core
H = 16
HD = 64
INNER = 4096
P = 128
DC = D // P    # 8
IC = INNER // P  # 32
EPS = 1e-5
NQT = 4        # key quarters
KQ = S // NQT  # 512 keys per quarter
NKT = KQ // P  # 4 key tiles of 128 per quarter

_BUILD_CACHE = {}

_DT = {"f32": dt.float32, "f32r": dt.float32r, "bf16": dt.bfloat16, "f16": dt.float16}


def _build(cfg, dbg=False):
    adt = _DT[cfg[0]]   # attention path: qkv/scores/PV operands
    pdt = _DT[cfg[1]]   # proj/fc/mlp path operands
    nc = bacc.Bacc("TRN2", target_bir_lowering=False, debug=False)
    if dbg:
        dbg_xln = nc.dram_tensor("dbg_xln", [NQT, P, DC, KQ], adt,
                                 kind="ExternalOutput")
        dbg_qt = nc.dram_tensor("dbg_qt", [P, DC, Q], adt,
                                kind="ExternalOutput")
        dbg_kt = nc.dram_tensor("dbg_kt", [NQT, P, DC, KQ], adt,
                                kind="ExternalOutput")
        dbg_acc = nc.dram_tensor("dbg_acc", [65, H, Q], dt.float32,
                                 kind="ExternalOutput")
        dbg_et = nc.dram_tensor("dbg_et", [NQT, P, NKT, KQ], adt,
                                kind="ExternalOutput")
        dbg_at2 = nc.dram_tensor("dbg_at2", [P, DC, Q], pdt,
                                 kind="ExternalOutput")
        dbg_h2 = nc.dram_tensor("dbg_h2", [P, DC, Q], dt.float32,
                                kind="ExternalOutput")
        dbg_h2n = nc.dram_tensor("dbg_h2n", [P, DC, Q], pdt,
                                 kind="ExternalOutput")
        dbg_g = nc.dram_tensor("dbg_g", [P, IC // 2, Q], pdt,
                               kind="ExternalOutput")

    hT = nc.dram_tensor("hT", [D, S], dt.float32, kind="ExternalInput")
    dmask = nc.dram_tensor("dmask", [P, NKT, P], adt, kind="ExternalInput")
    # weights arrive pre-tiled from the host in consumption order
    w_q = nc.dram_tensor("w_q", [DC, P, DC, P], adt, kind="ExternalInput")
    w_k = nc.dram_tensor("w_k", [DC, P, DC, P], adt, kind="ExternalInput")
    w_v = nc.dram_tensor("w_v", [2, P, DC, 512], adt, kind="ExternalInput")
    w_projr = nc.dram_tensor("w_projr", [DC, P, DC, P], pdt, kind="ExternalInput")
    w_fcr = nc.dram_tensor("w_fcr", [IC, P, DC, P], pdt, kind="ExternalInput")
    w_mlpr = nc.dram_tensor("w_mlpr", [2, DC, P, IC // 2, P], pdt, kind="ExternalInput")
    bq = nc.dram_tensor("bq", [P, DC], dt.float32, kind="ExternalInput")
    bk = nc.dram_tensor("bk", [P, DC], dt.float32, kind="ExternalInput")
    bv = nc.dram_tensor("bv", [HD, H], dt.float32, kind="ExternalInput")
    bproj = nc.dram_tensor("bproj", [P, DC], dt.float32, kind="ExternalInput")
    bfc = nc.dram_tensor("bfc", [P, IC], dt.float32, kind="ExternalInput")
    bmlp = nc.dram_tensor("bmlp", [P, DC], dt.float32, kind="ExternalInput")
    outT = nc.dram_tensor("outT", [D, Q], dt.float32, kind="ExternalOutput")

    hT_r = hT.rearrange("(c p) n -> p c n", p=P)

    with tile.TileContext(nc) as tc:
        with (
            tc.tile_pool(name="const", bufs=1) as const,
            tc.tile_pool(name="rows", bufs=2) as rows,
            tc.tile_pool(name="tmp", bufs=2) as tmp,
            tc.tile_pool(name="rowtmp", bufs=2) as rowtmp,
            tc.tile_pool(name="persist", bufs=1) as persist,
        ):
            ones_col = const.tile([P, 1], dt.float32)
            nc.vector.memset(ones_col[:], 1.0)
            ones_row = const.tile([1, P], dt.float32)
            nc.vector.memset(ones_row[:], 1.0)
            # bf16 ones: fp32 matmuls run LOW_HIGH two-pass at 4 cycles/row,
            # so the LN stats / row-broadcast matmuls use bf16 operands
            ones_col_bf = const.tile([P, 1], adt)
            nc.vector.memset(ones_col_bf[:], 1.0)
            ones_row_bf = const.tile([1, P], adt)
            nc.vector.memset(ones_row_bf[:], 1.0)
            eps_t = const.tile([1, 1], dt.float32)
            nc.vector.memset(eps_t[:], EPS)

            def load_pvec(t):
                s = const.tile(list(t.shape), dt.float32, tag=t.name)
                nc.sync.dma_start(s[:], t[:])
                return s

            bq_s, bk_s, bv_s = load_pvec(bq), load_pvec(bk), load_pvec(bv)
            bproj_s, bfc_s, bmlp_s = load_pvec(bproj), load_pvec(bfc), load_pvec(bmlp)

            h2 = persist.tile([P, DC, Q], dt.float32, tag="h2")
            hq_sb = persist.tile([P, DC, Q], dt.float32, tag="hq")

            # LN stats for a [P, DC, KQ] fp32 block resident in SBUF.
            # The block is first cast to bf16 into `xln` (scalar ACT copies);
            # stats read the raw-bf16 cast (partition-sums via bf16 PE
            # ones-matmuls at 1 cycle/row — fp32 MMs would cost 4x), and the
            # apply pass later overwrites xln in place. sq on GpSimd, row
            # math on DVE, sqrt on scalar, reciprocal via fast DVE approx.
            # Per-element bf16 rounding averages out over the 1024-wide sums.
            def ln_stats(get_chunk, xln, lnps, tag, rowtag="lnrow",
                         sq_dve=False):
                pss = lnps.tile([1, KQ], dt.float32, tag=rowtag)
                psq = lnps.tile([1, KQ], dt.float32, tag=rowtag)
                eng = nc.vector if sq_dve else nc.gpsimd
                for c in range(DC):
                    # cast off the scalar engine mid-attention: exp feeds PV
                    # there, while gpsimd has slack
                    if sq_dve:
                        nc.scalar.activation(xln[:, c, :], get_chunk(c),
                                             F.Copy)
                    else:
                        nc.gpsimd.tensor_copy(xln[:, c, :], get_chunk(c))
                    sq = tmp.tile([P, KQ], adt, tag="sq")
                    eng.tensor_tensor(sq[:], xln[:, c, :], xln[:, c, :],
                                      Alu.mult)
                    nc.tensor.matmul(pss[:], ones_col_bf[:], xln[:, c, :],
                                     start=(c == 0), stop=(c == DC - 1))
                    nc.tensor.matmul(psq[:], ones_col_bf[:], sq[:],
                                     start=(c == 0), stop=(c == DC - 1))
                mean = rows.tile([1, KQ], dt.float32, tag="mean")
                nc.vector.tensor_scalar_mul(mean[:], pss[:], 1.0 / D)
                msq = rowtmp.tile([1, KQ], dt.float32, tag="lnrow")
                nc.vector.tensor_tensor(msq[:], mean[:], mean[:], Alu.mult)
                var = rowtmp.tile([1, KQ], dt.float32, tag="lnrow")
                nc.vector.scalar_tensor_tensor(
                    var[:], psq[:], 1.0 / D, msq[:], Alu.mult, Alu.subtract
                )
                std = rowtmp.tile([1, KQ], dt.float32, tag="lnrow")
                nc.scalar.activation(std[:], var[:], F.Sqrt, bias=eps_t[:])
                rstd = rows.tile([1, KQ], dt.float32, tag="rstd")
                nc.vector.reciprocal_approx_fast(rstd[:], std[:])
                mean_bf = rows.tile([1, KQ], adt, tag="meanbf")
                nc.vector.tensor_copy(mean_bf[:], mean[:])
                rstd_bf = rows.tile([1, KQ], adt, tag="rstdbf")
                nc.vector.tensor_copy(rstd_bf[:], rstd[:])
                return mean_bf, rstd_bf

            # apply pass: xln = (x - mean) * rstd (gamma/beta folded into
            # the consuming weights host-side), written as `odt`.
            def ln_apply_start(mean, rstd, bbpool, bbtag):
                mb = bbpool.tile([P, KQ], dt.float32, tag=bbtag)
                rb = bbpool.tile([P, KQ], dt.float32, tag=bbtag)
                nc.tensor.matmul(mb[:], ones_row_bf[:], mean[:],
                                 start=True, stop=True)
                nc.tensor.matmul(rb[:], ones_row_bf[:], rstd[:],
                                 start=True, stop=True)
                return mb, rb

            def ln_apply_chunk(get_chunk, mb, rb, out, c, gp=False):
                eng = nc.gpsimd if gp else nc.vector
                xc = get_chunk(c)
                t1 = tmp.tile([P, KQ], dt.float32, tag="lnt1")
                eng.tensor_tensor(t1[:], xc, mb[:], Alu.subtract)
                eng.tensor_tensor(out[:, c, :], t1[:], rb[:], Alu.mult)

            def ln_apply(get_chunk, mean, rstd, out, bbpool, bbtag,
                         gp_chunks=0):
                mb, rb = ln_apply_start(mean, rstd, bbpool, bbtag)
                for c in range(DC):
                    ln_apply_chunk(get_chunk, mb, rb, out, c,
                                   gp=(c >= DC - gp_chunks))

            with (
                tc.tile_pool(name="attnsc", bufs=1) as attnsc,
                tc.tile_pool(name="hqp", bufs=1) as hqp,
                tc.tile_pool(name="xlnp", bufs=2) as xlnp,
                tc.tile_pool(name="wkv", bufs=4) as wkv,
                tc.tile_pool(name="wvp", bufs=2) as wvp,
                tc.tile_pool(name="etp", bufs=3) as etp,
                tc.tile_pool(name="scp", bufs=3, space="PSUM") as scp,
                tc.tile_pool(name="fillp", bufs=1, space="PSUM") as fillp,
                tc.tile_pool(name="pvps", bufs=2, space="PSUM") as pvps,
                tc.tile_pool(name="lnps", bufs=2, space="PSUM") as lnps,
            ):
                qt = attnsc.tile([P, DC, Q], adt, tag="qt")
                attn_acc = attnsc.tile([65, H, Q], dt.float32, tag="attn_acc")
                kt_sb = attnsc.tile([P, DC, KQ], adt, tag="kt")
                v_sb = attnsc.tile([P, NKT, H * 65], adt, tag="v")
                vview = v_sb[:].rearrange("p k (h x) -> p k h x", x=65)
                nc.vector.tensor_copy(
                    vview[:, :, :, 64:65],
                    ones_col[:].to_broadcast([P, NKT, H, 1]),
                )
                dmask_s = const.tile([P, NKT, P], adt, tag="dmask")
                nc.sync.dma_start(dmask_s[:], dmask[:])

                hquart = {}

                def load_quarter(q):
                    t = hqp.tile([P, DC, KQ], dt.float32, tag="hquart")
                    for c in range(DC):
                        nc.sync.dma_start(
                            t[:, c, :], hT_r[:, c, q * KQ:(q + 1) * KQ]
                        )
                    hquart[q] = t

                def hchunk(q):
                    return lambda c: hquart[q][:, c, :]

                stats = {}

                xln_t = {}

                def start_quarter(qq, sq_dve=False):
                    # allocate the bf16 tile, cast + stats into it; the
                    # apply pass later rewrites it in place
                    xln = xlnp.tile([P, DC, KQ], adt, tag="xln")
                    xln_t[qq] = xln
                    stats[qq] = ln_stats(hchunk(qq), xln, pvps, str(qq),
                                         rowtag="pv", sq_dve=sq_dve)
                    return xln

                def xchunk(xln):
                    return lambda c: xln[:, c, :]

                def g_k(xln, p, pool, tag, on_scalar=False):
                    wk_t = wkv.tile([P, DC, P], adt, tag="wkq")
                    nc.sync.dma_start(wk_t[:], w_k[p])
                    psk = pool.tile([P, KQ], dt.float32, tag=tag)
                    for c in range(DC):
                        nc.tensor.matmul(
                            psk[:], wk_t[:, c, :], xln[:, c, :],
                            start=(c == 0), stop=(c == DC - 1),
                        )
                        if c == DC - 1:
                            if on_scalar:
                                nc.scalar.activation(
                                    kt_sb[:, p, :], psk[:], F.Identity,
                                    bias=bk_s[:, p:p + 1],
                                )
                            else:
                                nc.vector.tensor_scalar_add(
                                    kt_sb[:, p, :], psk[:], bk_s[:, p:p + 1]
                                )
                        yield

                def g_v(xln, wv_t, vs, kt, pool, tag):
                    psv = pool.tile([P, KQ], dt.float32, tag=tag)
                    for c in range(DC):
                        nc.tensor.matmul(
                            psv[:], xln[:, c, kt * P:(kt + 1) * P],
                            wv_t[:, c, :],
                            start=(c == 0), stop=(c == DC - 1),
                        )
                        if c == DC - 1:
                            dst = v_sb[
                                :, kt, vs * 8 * 65:(vs + 1) * 8 * 65
                            ].rearrange("p (h x) -> p h x", x=65)[:, :, 0:64]
                            nc.vector.tensor_copy(
                                dst, psv[:].rearrange("p (h x) -> p h x", x=64)
                            )
                        yield

                def g_q(xln, q, p, pool, tag):
                    wq_t = wkv.tile([P, DC, P], adt, tag="wkq")
                    nc.sync.dma_start(wq_t[:], w_q[p])
                    psq_ = pool.tile([P, KQ], dt.float32, tag=tag)
                    for c in range(DC):
                        nc.tensor.matmul(
                            psq_[:, 0:P], wq_t[:, c, :], xln[:, c, 384:512],
                            start=(c == 0), stop=(c == DC - 1),
                        )
                        if c == DC - 1:
                            nc.vector.tensor_scalar_add(
                                qt[:, p, q * P:(q + 1) * P], psq_[:, 0:P],
                                bq_s[:, p:p + 1],
                            )
                        yield

                def run_all(gen):
                    for _ in gen:
                        pass

                attnT2 = attnsc.tile([P, DC, Q], pdt, tag="attnT2")

                def emit_norm(h):
                    # per-head softmax normalization, interleaved right
                    # after head h's last (q=0) PV accumulation
                    srow = rowtmp.tile([1, Q], dt.float32, tag="srow")
                    nc.vector.tensor_copy(srow[:], attn_acc[64:65, h, :])
                    rrow = rowtmp.tile([1, Q], dt.float32, tag="rrow")
                    nc.vector.reciprocal_approx_fast(rrow[:], srow[:])
                    rrow_bf = rowtmp.tile([1, Q], adt, tag="rrowbf")
                    nc.vector.tensor_copy(rrow_bf[:], rrow[:])
                    bc = lnps.tile([P, Q], dt.float32, tag="lnrow")
                    nc.tensor.matmul(
                        bc[0:64, :], ones_row_bf[0:1, 0:64], rrow_bf[:],
                        start=True, stop=True,
                    )
                    t1 = tmp.tile([HD, Q], dt.float32, tag="anorm")
                    nc.vector.tensor_tensor(
                        t1[:], attn_acc[0:64, h, :], bc[0:64, :], Alu.mult
                    )
                    off = 64 * (h // 8)
                    nc.vector.tensor_scalar_add(
                        attnT2[off:off + 64, h % 8, :], t1[:], bv_s[:, h:h + 1]
                    )

                def emit_proj(mo, h2bf):
                    wp_t = wkv.tile([P, DC, P], pdt, tag="wkq")
                    nc.sync.dma_start(wp_t[:], w_projr[mo])
                    psp = scp.tile([P, KQ], dt.float32, tag="sc")
                    for c in range(DC):
                        nc.tensor.matmul(
                            psp[:], wp_t[:, c, :], attnT2[:, c, :],
                            start=(c == 0), stop=(c == DC - 1),
                        )
                    nc.vector.scalar_tensor_tensor(
                        h2[:, mo, :], psp[:], bproj_s[:, mo:mo + 1],
                        hq_sb[:, mo, :], Alu.add, Alu.add,
                    )
                    # bf16 mirror feeds the LN2 stats matmuls + apply
                    nc.scalar.activation(h2bf[:, mo, :], h2[:, mo, :], F.Copy)

                et_t = {}

                def emit_scores_pair(q, i, NQ, qsl, pull):
                    # Heads 2i / 2i+1 live on partition halves 0:64 / 64:128
                    # of chunk i. Their K=64 score MMs auto-derive
                    # tile_position (0,0) / (64,0); issued back-to-back they
                    # run CONCURRENTLY on disjoint PE row groups (~2x).
                    he, ho = 2 * i, 2 * i + 1
                    ete = etp.tile([P, NKT, KQ], adt, tag="et")
                    eto = etp.tile([P, NKT, KQ], adt, tag="et")
                    et_t[he], et_t[ho] = ete, eto
                    if NQ <= 256:
                        # [P, 2, NQ] fits one PSUM bank: pair the score MMs
                        # and halve the exp-ACT count
                        for half in range(2):
                            pse = scp.tile([P, 2, 256], dt.float32, tag="sc")
                            pso = scp.tile([P, 2, 256], dt.float32, tag="sc")
                            for k2 in range(2):
                                kt = half * 2 + k2
                                nc.tensor.matmul(
                                    pse[:, k2, 0:NQ],
                                    kt_sb[0:64, i, kt * P:(kt + 1) * P],
                                    qt[0:64, i, qsl],
                                    start=True, stop=True,
                                )
                                nc.tensor.matmul(
                                    pso[:, k2, 0:NQ],
                                    kt_sb[64:128, i, kt * P:(kt + 1) * P],
                                    qt[64:128, i, qsl],
                                    start=True, stop=True,
                                )
                                pull()
                            for et, psc in ((ete, pse), (eto, pso)):
                                nc.scalar.activation(
                                    et[:, half * 2:half * 2 + 2, 0:NQ],
                                    psc[:, :, 0:NQ], F.Exp, scale=0.125,
                                )
                                nc.gpsimd.tensor_tensor(
                                    et[:, half * 2:half * 2 + 2, 0:P],
                                    et[:, half * 2:half * 2 + 2, 0:P],
                                    dmask_s[:, half * 2:half * 2 + 2, :],
                                    Alu.mult,
                                )
                                pull()
                            pull()
                        return
                    for kt in range(NKT):
                        pse = scp.tile([P, KQ], dt.float32, tag="sc")
                        pso = scp.tile([P, KQ], dt.float32, tag="sc")
                        nc.tensor.matmul(
                            pse[:, 0:NQ],
                            kt_sb[0:64, i, kt * P:(kt + 1) * P],
                            qt[0:64, i, qsl],
                            start=True, stop=True,
                        )
                        nc.tensor.matmul(
                            pso[:, 0:NQ],
                            kt_sb[64:128, i, kt * P:(kt + 1) * P],
                            qt[64:128, i, qsl],
                            start=True, stop=True,
                        )
                        pull()
                        nc.scalar.activation(
                            ete[:, kt, 0:NQ], pse[:, 0:NQ], F.Exp, scale=0.125,
                        )
                        nc.scalar.activation(
                            eto[:, kt, 0:NQ], pso[:, 0:NQ], F.Exp, scale=0.125,
                        )
                        if kt % 2 == 1:
                            # multiplicative causal mask on the diagonal
                            # query tile (first 128 columns of the window)
                            nc.gpsimd.tensor_tensor(
                                ete[:, kt - 1:kt + 1, 0:P],
                                ete[:, kt - 1:kt + 1, 0:P],
                                dmask_s[:, kt - 1:kt + 1, :], Alu.mult,
                            )
                            nc.gpsimd.tensor_tensor(
                                eto[:, kt - 1:kt + 1, 0:P],
                                eto[:, kt - 1:kt + 1, 0:P],
                                dmask_s[:, kt - 1:kt + 1, :], Alu.mult,
                            )
                        pull()

                def emit_pv(q, h, NQ, qsl, pull=None):
                    et = et_t.pop(h)
                    pa = pvps.tile([65, KQ], dt.float32, tag="pv")
                    for kt in range(NKT):
                        nc.tensor.matmul(
                            pa[:, qsl], v_sb[:, kt, h * 65:h * 65 + 65],
                            et[:, kt, 0:NQ],
                            start=(kt == 0), stop=(kt == NKT - 1),
                        )
                        if pull is not None:
                            pull()
                    nc.scalar.activation(
                        attn_acc[:, h, q * P:(q + 1) * P],
                        pa[:, q * P:(q + 1) * P], F.Copy,
                    )
                    if q < NQT - 1:
                        nc.vector.tensor_tensor(
                            attn_acc[:, h, (q + 1) * P:],
                            attn_acc[:, h, (q + 1) * P:],
                            pa[:, (q + 1) * P:], Alu.add,
                        )
                    if q == 0:
                        emit_norm(h)

                # prologue: quarter 3 LN fully, eagerly; own-rows DMA for
                # the residual path is issued after the critical q3 data
                load_quarter(3)
                # V weights are quarter-invariant: fetch once, keep resident
                wv0 = wvp.tile([P, DC, 512], adt, tag="wv")
                nc.sync.dma_start(wv0[:], w_v[0])
                wv1 = wvp.tile([P, DC, 512], adt, tag="wv")
                nc.sync.dma_start(wv1[:], w_v[1])
                xln3 = start_quarter(3, sq_dve=True)
                mean3, rstd3 = stats.pop(3)
                mb3, rb3 = ln_apply_start(mean3, rstd3, lnps, "lnrow")
                for c in range(DC):
                    ln_apply_chunk(xchunk(xln3), mb3, rb3, xln3, c)

                for q in range(NQT - 1, -1, -1):
                    if q == 1:
                        # residual own-rows, needed only by the proj phase
                        for c in range(DC):
                            nc.sync.dma_start(
                                hq_sb[:, c, :],
                                hT_r[:, c, :].rearrange(
                                    "p (j n) -> p j n", n=KQ
                                )[:, :, 384:512],
                            )
                    xln = xln_t.pop(q)
                    nxt = None
                    if q > 0:
                        load_quarter(q - 1)
                        xln_n = start_quarter(q - 1)
                        nxt = (*stats.pop(q - 1), xln_n)

                    # preamble: K0, V(vs0) x4, Q0..Q3 dense (scp slots)
                    run_all(g_k(xln, 0, scp, "sc"))
                    for kt in range(NKT):
                        run_all(g_v(xln, wv0, 0, kt, scp, "sc"))
                    run_all(g_k(xln, 1, scp, "sc"))
                    for p in range(4):
                        run_all(g_q(xln, q, p, scp, "sc"))

                    # fine-grained fill queue: two MMs pulled after every
                    # score so the PE never drains while exp catches up
                    def fill_iter():
                        yield from g_k(xln, 2, fillp, "mm", on_scalar=(q > 0))
                        for kt in range(2):
                            yield from g_v(xln, wv1, 1, kt, fillp, "mm")
                        yield from g_k(xln, 3, fillp, "mm", on_scalar=(q > 0))
                        for kt in range(2, 4):
                            yield from g_v(xln, wv1, 1, kt, fillp, "mm")
                        yield from g_k(xln, 4, fillp, "mm", on_scalar=(q > 0))
                        yield from g_q(xln, q, 4, fillp, "mm")
                        yield from g_k(xln, 5, fillp, "mm", on_scalar=(q > 0))
                        yield from g_q(xln, q, 5, fillp, "mm")
                        yield from g_k(xln, 6, fillp, "mm", on_scalar=(q > 0))
                        yield from g_q(xln, q, 6, fillp, "mm")
                        yield from g_k(xln, 7, fillp, "mm", on_scalar=(q > 0))
                        yield from g_q(xln, q, 7, fillp, "mm")

                    fq = fill_iter()

                    def pull():
                        next(fq, None)

                    NQ = (NQT - q) * P
                    qsl = slice(q * P, Q)
                    # pv stagger keeps at most 3 et tiles live: iteration i
                    # consumes pair i-1's odd head and pair i's even head
                    for i in range(H // 2):
                        if i == 0 and nxt is not None:
                            mb_n, rb_n = ln_apply_start(nxt[0], nxt[1],
                                                        lnps, "lnrow")
                        emit_scores_pair(q, i, NQ, qsl, pull)
                        if i > 0:
                            emit_pv(q, 2 * i - 1, NQ, qsl, pull)
                        emit_pv(q, 2 * i, NQ, qsl, pull)
                        if nxt is not None:
                            ln_apply_chunk(xchunk(nxt[2]), mb_n, rb_n,
                                           nxt[2], i)
                    run_all(fq)
                    emit_pv(q, H - 1, NQ, qsl)

                if dbg:
                    nc.sync.dma_start(dbg_qt[:], qt[:])
                    nc.sync.dma_start(dbg_acc[:], attn_acc[:])

                # h2 bf16 mirror reuses the xln slot (dead after quarter 0)
                h2bf = xlnp.tile([P, DC, Q], adt, tag="xln")
                # LN2 stats interleave into the proj loop (lagged 2 so the
                # first stats MM doesn't stall on the attention-tail scalar
                # backlog feeding the h2bf casts)
                pss2 = pvps.tile([1, KQ], dt.float32, tag="pv")
                psq2 = pvps.tile([1, KQ], dt.float32, tag="pv")

                def ln2_stats(mo):
                    sq2 = tmp.tile([P, KQ], adt, tag="sq")
                    nc.gpsimd.tensor_tensor(
                        sq2[:], h2bf[:, mo, :], h2bf[:, mo, :], Alu.mult
                    )
                    nc.tensor.matmul(pss2[:], ones_col_bf[:], h2bf[:, mo, :],
                                     start=(mo == 0), stop=(mo == DC - 1))
                    nc.tensor.matmul(psq2[:], ones_col_bf[:], sq2[:],
                                     start=(mo == 0), stop=(mo == DC - 1))

                for mo in range(DC):
                    emit_proj(mo, h2bf)
                    if mo >= 2:
                        ln2_stats(mo - 2)
                ln2_stats(DC - 2)
                ln2_stats(DC - 1)
                mean2 = rows.tile([1, KQ], dt.float32, tag="mean")
                nc.vector.tensor_scalar_mul(mean2[:], pss2[:], 1.0 / D)
                msq2 = rowtmp.tile([1, KQ], dt.float32, tag="lnrow")
                nc.vector.tensor_tensor(msq2[:], mean2[:], mean2[:], Alu.mult)
                var2 = rowtmp.tile([1, KQ], dt.float32, tag="lnrow")
                nc.vector.scalar_tensor_tensor(
                    var2[:], psq2[:], 1.0 / D, msq2[:], Alu.mult, Alu.subtract
                )
                std2 = rowtmp.tile([1, KQ], dt.float32, tag="lnrow")
                nc.scalar.activation(std2[:], var2[:], F.Sqrt, bias=eps_t[:])
                rstd2 = rows.tile([1, KQ], dt.float32, tag="rstd")
                nc.vector.reciprocal_approx_fast(rstd2[:], std2[:])
                mean2_bf = rows.tile([1, KQ], adt, tag="meanbf")
                nc.vector.tensor_copy(mean2_bf[:], mean2[:])
                rstd2_bf = rows.tile([1, KQ], adt, tag="rstdbf")
                nc.vector.tensor_copy(rstd2_bf[:], rstd2[:])
                if dbg:
                    nc.sync.dma_start(dbg_at2[:], attnT2[:])

            # ---- LN2 / fc+gelu / mlp + residual ----
            with (
                tc.tile_pool(name="mlpsc", bufs=1) as mlpsc,
                tc.tile_pool(name="wfcs", bufs=4) as wfcs,
                tc.tile_pool(name="wmlps", bufs=4) as wmlps,
                tc.tile_pool(name="psfc", bufs=2, space="PSUM") as psfc,
                tc.tile_pool(name="psm", bufs=2, space="PSUM") as psm,
                tc.tile_pool(name="lnps2", bufs=2, space="PSUM") as lnps2,
            ):
                h2c = lambda c: h2[:, c, :]
                h2n = mlpsc.tile([P, DC, Q], pdt, tag="h2n")
                ln_apply(h2c, mean2_bf, rstd2_bf, h2n, lnps2, "lnbb")
                if dbg:
                    nc.sync.dma_start(dbg_h2[:], h2[:])
                    nc.sync.dma_start(dbg_h2n[:], h2n[:])
                y2 = mlpsc.tile([P, DC, Q], dt.float32, tag="y2")
                g_half = mlpsc.tile([P, IC // 2, Q], pdt, tag="g")
                for ih in range(2):
                    for m in range(IC // 2):
                        mg = ih * (IC // 2) + m
                        wfc_t = wfcs.tile([P, DC, P], pdt, tag="wfc")
                        nc.sync.dma_start(wfc_t[:], w_fcr[mg])
                        psf = psfc.tile([P, Q], dt.float32, tag="fc")
                        for c in range(DC):
                            nc.tensor.matmul(
                                psf[:], wfc_t[:, c, :], h2n[:, c, :],
                                start=(c == 0), stop=(c == DC - 1),
                            )
                        nc.scalar.activation(
                            g_half[:, m, :], psf[:], F.Gelu,
                            bias=bfc_s[:, mg:mg + 1],
                        )
                    if dbg and ih == 0:
                        nc.sync.dma_start(dbg_g[:], g_half[:])
                    for mo in range(DC):
                        wm_t = wmlps.tile([P, IC // 2, P], pdt, tag="wmlp")
                        nc.sync.dma_start(wm_t[:], w_mlpr[ih, mo])
                        psm_ = psm.tile([P, Q], dt.float32, tag="mm2")
                        for c in range(IC // 2):
                            nc.tensor.matmul(
                                psm_[:], wm_t[:, c, :], g_half[:, c, :],
                                start=(c == 0), stop=(c == IC // 2 - 1),
                            )
                        if ih == 0:
                            nc.vector.tensor_copy(y2[:, mo, :], psm_[:])
                        else:
                            ot = tmp.tile([P, Q], dt.float32, tag="anorm")
                            nc.vector.tensor_tensor(
                                ot[:], y2[:, mo, :], psm_[:], Alu.add
                            )
                            nc.vector.scalar_tensor_tensor(
                                ot[:], ot[:], bmlp_s[:, mo:mo + 1],
                                h2[:, mo, :], Alu.add, Alu.add,
                            )
                            nc.sync.dma_start(
                                outT.rearrange("(c p) n -> p c n", p=P)[:, mo, :],
                                ot[:],
                            )

    nc.compile()
    return nc


def _get_nc(cfg):
    if cfg not in _BUILD_CACHE:
        _BUILD_CACHE[cfg] = _build(cfg)
    return _BUILD_CACHE[cfg]


def _np_dt(name):
    if name == "bf16":
        import ml_dtypes
        return ml_dtypes.bfloat16
    if name == "f16":
        return np.float16
    return np.float32


def _perm_for_core(a):
    """Key order per quarter: non-own keys in natural order, own 128 last."""
    perm = []
    for q in range(NQT):
        base = q * KQ
        own = np.arange(base + 128 * a, base + 128 * a + P)
        others = np.setdiff1d(np.arange(base, base + KQ), own)
        perm.append(np.concatenate([others, own]))
    return np.concatenate(perm)


def _prep_in_maps(inputs, cfg):
    adt_np, pdt_np = _np_dt(cfg[0]), _np_dt(cfg[1])
    h = np.asarray(inputs["hidden_states"], dtype=np.float32)
    w_qkv = np.asarray(inputs["w_qkv"], np.float32)
    b_qkv = np.asarray(inputs["b_qkv"], np.float32)
    g1 = np.asarray(inputs["g1"], np.float32)
    be1 = np.asarray(inputs["be1"], np.float32)
    g2 = np.asarray(inputs["g2"], np.float32)
    be2 = np.asarray(inputs["be2"], np.float32)

    # fold LN1 gamma/beta into the qkv weights and biases
    w_qkv_f = w_qkv * g1[:, None]
    b_qkv_f = b_qkv + be1 @ w_qkv
    w_fc = np.asarray(inputs["w_fc"], np.float32)
    b_fc = np.asarray(inputs["b_fc"], np.float32)
    w_fc_f = w_fc * g2[:, None]
    b_fc_f = b_fc + be2 @ w_fc

    def chunk_w(w, p=P):  # [Din, N] -> [p, Din//p, N]
        return np.ascontiguousarray(w.reshape(-1, p, w.shape[1]).transpose(1, 0, 2))

    def pvec(v, p=P):  # [n*p] -> [p, n]
        return np.ascontiguousarray(v.reshape(-1, p).T)

    def mslice(a, nsl):  # [p, c, n] -> [n//nsl, p, c, nsl]
        p, c, n = a.shape
        return np.ascontiguousarray(
            a.reshape(p, c, n // nsl, nsl).transpose(2, 0, 1, 3)
        )

    wq = mslice(chunk_w(w_qkv_f[:, 0:D]), P)
    wk = mslice(chunk_w(w_qkv_f[:, D:2 * D]), P)
    wv = mslice(chunk_w(w_qkv_f[:, 2 * D:3 * D]), 512)

    # proj with head pairs (c, c+8) stacked on the 128 contraction rows
    w_proj = np.asarray(inputs["w_proj"], np.float32)
    w2 = w_proj.reshape(H, HD, D)
    wp_t = np.concatenate([w2[0:8], w2[8:16]], axis=1)  # [8, 128, 1024]
    wp = np.ascontiguousarray(
        wp_t.reshape(DC, P, DC, P).transpose(2, 1, 0, 3)
    )

    wfc = mslice(chunk_w(w_fc_f), P)
    wm = chunk_w(np.asarray(inputs["w_mlp"], np.float32))  # [128, 32, 1024]
    wmlp = np.ascontiguousarray(
        wm.reshape(P, 2, IC // 2, DC, P).transpose(1, 3, 0, 2, 4)
    )

    shared = {
        "w_q": wq.astype(adt_np), "w_k": wk.astype(adt_np),
        "w_v": wv.astype(adt_np), "w_projr": wp.astype(pdt_np),
        "w_fcr": wfc.astype(pdt_np), "w_mlpr": wmlp.astype(pdt_np),
        "bq": pvec(b_qkv_f[0:D]),
        "bk": pvec(b_qkv_f[D:2 * D]),
        "bv": pvec(b_qkv_f[2 * D:3 * D], p=HD),
        "bproj": pvec(np.asarray(inputs["b_proj"], np.float32)),
        "bfc": pvec(b_fc_f),
        "bmlp": pvec(np.asarray(inputs["b_mlp"], np.float32)),
    }
    in_maps = []
    for core in range(8):
        b, a = core // 4, core % 4
        perm = _perm_for_core(a)
        hrot = h[b, perm]
        # multiplicative 0/1 mask on exp'd scores for the diagonal query
        # tile: key position j=kt*128+p in the quarter vs query row r.
        pp = np.arange(P)
        dm = np.zeros((P, NKT, P), np.float32)
        for kt in range(NKT - 1):
            dm[:, kt, :] = ((kt * P + pp) < (P * a)).astype(np.float32)[:, None]
        dm[:, NKT - 1, :] = (pp[:, None] <= pp[None, :]).astype(np.float32)
        in_maps.append(
            dict(
                shared,
                hT=np.ascontiguousarray(hrot.T),
                dmask=dm.astype(adt_np),
            )
        )
    return in_maps


def _stitch(results):
    out = np.empty((2, S, D), dtype=np.float32)
    for core in range(8):
        b, a = core // 4, core % 4
        r = results[core]["outT"].T  # [512, D]: cols j*128+p -> row 512j+128a+p
        for j in range(NQT):
            out[b, j * KQ + P * a: j * KQ + P * a + P] = r[j * P:(j + 1) * P]
    return out


def run(inputs, cfg=("bf16", "bf16"), trace=False, trace_cores=None):
    nc = _get_nc(cfg)
    in_maps = _prep_in_maps(inputs, cfg)
    res = bass_utils.run_bass_kernel_spmd(
        nc, in_maps, core_ids=list(range(8)), trace=trace, trace_cores=trace_cores
    )
    return _stitch(res.results), res


def kernel(**inputs) -> np.ndarray:
    out, _ = run(inputs, cfg=("bf16", "bf16"))
    return out



# revision 60
# speedup vs baseline: 1.4826x; 1.2634x over previous
"""GPT2 block kernel for 8 TRN2 NeuronCores (Bass/Tile, SPMD).

Sharding: the 4096 rows (batch*seq) are split 8 ways -> 512 rows/core
(4 cores per batch element). Core (b, a) owns query blocks {4j + a}
(128 rows each, one per key-quarter j) of batch b. Each core
redundantly computes K,V for its batch, but scores/PV/exp only for
(quarter q) x (own query tiles j >= q) -- 62.5% of the full rectangle.
Quarters are processed in DESCENDING order so query tile j (produced
from quarter j's LayerNorm output) exists before quarters q < j consume
it. Zero collectives.

Per-core key permutation (host side): within each quarter, the core's
own 128 rows are moved to the last 128 key positions, so the Q-proj
input is always xln[:, :, 384:512] -- uniform addresses across cores;
the causal mask becomes a per-core data tensor (multiplicative 0/1 on
the exp'd scores, applied by the otherwise-idle GpSimd engine).

LN gamma/beta are folded into the consuming weights host-side, exp is
batched into [128, 2, N] activations, reciprocals use the fast approx
DVE op, and proj packs head pairs to a full K=128 contraction.
"""

import numpy as np
import sys

sys.path.insert(0, "/opt/trn_rl_repo")

import concourse.bacc as bacc
import concourse.mybir as mybir
import concourse.tile as tile
from concourse import bass_utils

dt = mybir.dt
F = mybir.ActivationFunctionType
Alu = mybir.AluOpType

D = 1024
S = 2048
Q = 512        # own rows per core
H = 16
HD = 64
INNER = 4096
P = 128
DC = D // P    # 8
IC = INNER // P  # 32
EPS = 1e-5
NQT = 4        # key quarters
KQ = S // NQT  # 512 keys per quarter
NKT = KQ // P  # 4 key tiles of 128 per quarter

_BUILD_CACHE = {}

_DT = {"f32": dt.float32, "f32r": dt.float32r, "bf16": dt.bfloat16, "f16": dt.float16}


def _build(cfg, dbg=False):
    adt = _DT[cfg[0]]   # attention path: qkv/scores/PV operands
    pdt = _DT[cfg[1]]   # proj/fc/mlp path operands
    nc = bacc.Bacc("TRN2", target_bir_lowering=False, debug=False)
    if dbg:
        dbg_xln = nc.dram_tensor("dbg_xln", [NQT, P, DC, KQ], adt,
                                 kind="ExternalOutput")
        dbg_qt = nc.dram_tensor("dbg_qt", [P, DC, Q], adt,
                                kind="ExternalOutput")
        dbg_kt = nc.dram_tensor("dbg_kt", [NQT, P, DC, KQ], adt,
                                kind="ExternalOutput")
        dbg_acc = nc.dram_tensor("dbg_acc", [65, H, Q], dt.float32,
                                 kind="ExternalOutput")
        dbg_et = nc.dram_tensor("dbg_et", [NQT, P, NKT, KQ], adt,
                                kind="ExternalOutput")
        dbg_at2 = nc.dram_tensor("dbg_at2", [P, DC, Q], pdt,
                                 kind="ExternalOutput")
        dbg_h2 = nc.dram_tensor("dbg_h2", [P, DC, Q], dt.float32,
                                kind="ExternalOutput")
        dbg_h2n = nc.dram_tensor("dbg_h2n", [P, DC, Q], pdt,
                                 kind="ExternalOutput")
        dbg_g = nc.dram_tensor("dbg_g", [P, IC // 2, Q], pdt,
                               kind="ExternalOutput")

    hT = nc.dram_tensor("hT", [D, S], dt.float32, kind="ExternalInput")
    dmask = nc.dram_tensor("dmask", [P, NKT, P], adt, kind="ExternalInput")
    # weights arrive pre-tiled from the host in consumption order
    w_q = nc.dram_tensor("w_q", [DC, P, DC, P], adt, kind="ExternalInput")
    w_k = nc.dram_tensor("w_k", [DC, P, DC, P], adt, kind="ExternalInput")
    w_v = nc.dram_tensor("w_v", [2, P, DC, 512], adt, kind="ExternalInput")
    w_projr = nc.dram_tensor("w_projr", [DC, P, DC, P], pdt, kind="ExternalInput")
    w_fcr = nc.dram_tensor("w_fcr", [IC, P, DC, P], pdt, kind="ExternalInput")
    w_mlpr = nc.dram_tensor("w_mlpr", [2, DC, P, IC // 2, P], pdt, kind="ExternalInput")
    bq = nc.dram_tensor("bq", [P, DC], dt.float32, kind="ExternalInput")
    bk = nc.dram_tensor("bk", [P, DC], dt.float32, kind="ExternalInput")
    bv = nc.dram_tensor("bv", [HD, H], dt.float32, kind="ExternalInput")
    bproj = nc.dram_tensor("bproj", [P, DC], dt.float32, kind="ExternalInput")
    bfc = nc.dram_tensor("bfc", [P, IC], dt.float32, kind="ExternalInput")
    bmlp = nc.dram_tensor("bmlp", [P, DC], dt.float32, kind="ExternalInput")
    outT = nc.dram_tensor("outT", [D, Q], dt.float32, kind="ExternalOutput")

    hT_r = hT.rearrange("(c p) n -> p c n", p=P)

    with tile.TileContext(nc) as tc:
        with (
            tc.tile_pool(name="const", bufs=1) as const,
            tc.tile_pool(name="rows", bufs=2) as rows,
            tc.tile_pool(name="tmp", bufs=2) as tmp,
            tc.tile_pool(name="rowtmp", bufs=2) as rowtmp,
            tc.tile_pool(name="persist", bufs=1) as persist,
        ):
            ones_col = const.tile([P, 1], dt.float32)
            nc.vector.memset(ones_col[:], 1.0)
            ones_row = const.tile([1, P], dt.float32)
            nc.vector.memset(ones_row[:], 1.0)
            # bf16 ones: fp32 matmuls run LOW_HIGH two-pass at 4 cycles/row,
            # so the LN stats / row-broadcast matmuls use bf16 operands
            ones_col_bf = const.tile([P, 1], adt)
            nc.vector.memset(ones_col_bf[:], 1.0)
            ones_row_bf = const.tile([1, P], adt)
            nc.vector.memset(ones_row_bf[:], 1.0)
            eps_t = const.tile([1, 1], dt.float32)
            nc.vector.memset(eps_t[:], EPS)

            def load_pvec(t):
                s = const.tile(list(t.shape), dt.float32, tag=t.name)
                nc.sync.dma_start(s[:], t[:])
                return s

            bq_s, bk_s, bv_s = load_pvec(bq), load_pvec(bk), load_pvec(bv)
            bproj_s, bfc_s, bmlp_s = load_pvec(bproj), load_pvec(bfc), load_pvec(bmlp)

            h2 = persist.tile([P, DC, Q], dt.float32, tag="h2")
            hq_sb = persist.tile([P, DC, Q], dt.float32, tag="hq")

            # LN stats for a [P, DC, KQ] fp32 block resident in SBUF.
            # The block is first cast to bf16 into `xln` (scalar ACT copies);
            # stats read the raw-bf16 cast (partition-sums via bf16 PE
            # ones-matmuls at 1 cycle/row — fp32 MMs would cost 4x), and the
            # apply pass later overwrites xln in place. sq on GpSimd, row
            # math on DVE, sqrt on scalar, reciprocal via fast DVE approx.
            # Per-element bf16 rounding averages out over the 1024-wide sums.
            def ln_stats(get_chunk, xln, lnps, tag, rowtag="lnrow",
                         sq_dve=False):
                pss = lnps.tile([1, KQ], dt.float32, tag=rowtag)
                psq = lnps.tile([1, KQ], dt.float32, tag=rowtag)
                eng = nc.vector if sq_dve else nc.gpsimd
                for c in range(DC):
                    nc.scalar.activation(xln[:, c, :], get_chunk(c), F.Copy)
                    sq = tmp.tile([P, KQ], adt, tag="sq")
                    eng.tensor_tensor(sq[:], xln[:, c, :], xln[:, c, :],
                                      Alu.mult)
                    nc.tensor.matmul(pss[:], ones_col_bf[:], xln[:, c, :],
                                     start=(c == 0), stop=(c == DC - 1))
                    nc.tensor.matmul(psq[:], ones_col_bf[:], sq[:],
                                     start=(c == 0), stop=(c == DC - 1))
                mean = rows.tile([1, KQ], dt.float32, tag="mean")
                nc.vector.tensor_scalar_mul(mean[:], pss[:], 1.0 / D)
                msq = rowtmp.tile([1, KQ], dt.float32, tag="lnrow")
                nc.vector.tensor_tensor(msq[:], mean[:], mean[:], Alu.mult)
                var = rowtmp.tile([1, KQ], dt.float32, tag="lnrow")
                nc.vector.scalar_tensor_tensor(
                    var[:], psq[:], 1.0 / D, msq[:], Alu.mult, Alu.subtract
                )
                std = rowtmp.tile([1, KQ], dt.float32, tag="lnrow")
                nc.scalar.activation(std[:], var[:], F.Sqrt, bias=eps_t[:])
                rstd = rows.tile([1, KQ], dt.float32, tag="rstd")
                nc.vector.reciprocal_approx_fast(rstd[:], std[:])
                mean_bf = rows.tile([1, KQ], adt, tag="meanbf")
                nc.vector.tensor_copy(mean_bf[:], mean[:])
                rstd_bf = rows.tile([1, KQ], adt, tag="rstdbf")
                nc.vector.tensor_copy(rstd_bf[:], rstd[:])
                return mean_bf, rstd_bf

            # apply pass: xln = (x - mean) * rstd (gamma/beta folded into
            # the consuming weights host-side), written as `odt`.
            def ln_apply_start(mean, rstd, bbpool, bbtag):
                mb = bbpool.tile([P, KQ], dt.float32, tag=bbtag)
                rb = bbpool.tile([P, KQ], dt.float32, tag=bbtag)
                nc.tensor.matmul(mb[:], ones_row_bf[:], mean[:],
                                 start=True, stop=True)
                nc.tensor.matmul(rb[:], ones_row_bf[:], rstd[:],
                                 start=True, stop=True)
                return mb, rb

            def ln_apply_chunk(get_chunk, mb, rb, out, c, gp=False):
                eng = nc.gpsimd if gp else nc.vector
                xc = get_chunk(c)
                t1 = tmp.tile([P, KQ], dt.float32, tag="lnt1")
                eng.tensor_tensor(t1[:], xc, mb[:], Alu.subtract)
                eng.tensor_tensor(out[:, c, :], t1[:], rb[:], Alu.mult)

            def ln_apply(get_chunk, mean, rstd, out, bbpool, bbtag,
                         gp_chunks=0):
                mb, rb = ln_apply_start(mean, rstd, bbpool, bbtag)
                for c in range(DC):
                    ln_apply_chunk(get_chunk, mb, rb, out, c,
                                   gp=(c >= DC - gp_chunks))

            with (
                tc.tile_pool(name="attnsc", bufs=1) as attnsc,
                tc.tile_pool(name="hqp", bufs=1) as hqp,
                tc.tile_pool(name="xlnp", bufs=2) as xlnp,
                tc.tile_pool(name="wkv", bufs=4) as wkv,
                tc.tile_pool(name="wvp", bufs=2) as wvp,
                tc.tile_pool(name="etp", bufs=3) as etp,
                tc.tile_pool(name="scp", bufs=3, space="PSUM") as scp,
                tc.tile_pool(name="fillp", bufs=1, space="PSUM") as fillp,
                tc.tile_pool(name="pvps", bufs=2, space="PSUM") as pvps,
                tc.tile_pool(name="lnps", bufs=2, space="PSUM") as lnps,
            ):
                qt = attnsc.tile([P, DC, Q], adt, tag="qt")
                attn_acc = attnsc.tile([65, H, Q], dt.float32, tag="attn_acc")
                kt_sb = attnsc.tile([P, DC, KQ], adt, tag="kt")
                v_sb = attnsc.tile([P, NKT, H * 65], adt, tag="v")
                vview = v_sb[:].rearrange("p k (h x) -> p k h x", x=65)
                nc.vector.tensor_copy(
                    vview[:, :, :, 64:65],
                    ones_col[:].to_broadcast([P, NKT, H, 1]),
                )
                dmask_s = const.tile([P, NKT, P], adt, tag="dmask")
                nc.sync.dma_start(dmask_s[:], dmask[:])

                hquart = {}

                def load_quarter(q):
                    t = hqp.tile([P, DC, KQ], dt.float32, tag="hquart")
                    for c in range(DC):
                        nc.sync.dma_start(
                            t[:, c, :], hT_r[:, c, q * KQ:(q + 1) * KQ]
                        )
                    hquart[q] = t

                def hchunk(q):
                    return lambda c: hquart[q][:, c, :]

                stats = {}

                xln_t = {}

                def start_quarter(qq, sq_dve=False):
                    # allocate the bf16 tile, cast + stats into it; the
                    # apply pass later rewrites it in place
                    xln = xlnp.tile([P, DC, KQ], adt, tag="xln")
                    xln_t[qq] = xln
                    stats[qq] = ln_stats(hchunk(qq), xln, pvps, str(qq),
                                         rowtag="pv", sq_dve=sq_dve)
                    return xln

                def xchunk(xln):
                    return lambda c: xln[:, c, :]

                def g_k(xln, p, pool, tag, on_scalar=False):
                    wk_t = wkv.tile([P, DC, P], adt, tag="wkq")
                    nc.sync.dma_start(wk_t[:], w_k[p])
                    psk = pool.tile([P, KQ], dt.float32, tag=tag)
                    for c in range(DC):
                        nc.tensor.matmul(
                            psk[:], wk_t[:, c, :], xln[:, c, :],
                            start=(c == 0), stop=(c == DC - 1),
                        )
                        if c == DC - 1:
                            if on_scalar:
                                nc.scalar.activation(
                                    kt_sb[:, p, :], psk[:], F.Identity,
                                    bias=bk_s[:, p:p + 1],
                                )
                            else:
                                nc.vector.tensor_scalar_add(
                                    kt_sb[:, p, :], psk[:], bk_s[:, p:p + 1]
                                )
                        yield

                def g_v(xln, wv_t, vs, kt, pool, tag):
                    psv = pool.tile([P, KQ], dt.float32, tag=tag)
                    for c in range(DC):
                        nc.tensor.matmul(
                            psv[:], xln[:, c, kt * P:(kt + 1) * P],
                            wv_t[:, c, :],
                            start=(c == 0), stop=(c == DC - 1),
                        )
                        if c == DC - 1:
                            dst = v_sb[
                                :, kt, vs * 8 * 65:(vs + 1) * 8 * 65
                            ].rearrange("p (h x) -> p h x", x=65)[:, :, 0:64]
                            nc.vector.tensor_copy(
                                dst, psv[:].rearrange("p (h x) -> p h x", x=64)
                            )
                        yield

                def g_q(xln, q, p, pool, tag):
                    wq_t = wkv.tile([P, DC, P], adt, tag="wkq")
                    nc.sync.dma_start(wq_t[:], w_q[p])
                    psq_ = pool.tile([P, KQ], dt.float32, tag=tag)
                    for c in range(DC):
                        nc.tensor.matmul(
                            psq_[:, 0:P], wq_t[:, c, :], xln[:, c, 384:512],
                            start=(c == 0), stop=(c == DC - 1),
                        )
                        if c == DC - 1:
                            nc.vector.tensor_scalar_add(
                                qt[:, p, q * P:(q + 1) * P], psq_[:, 0:P],
                                bq_s[:, p:p + 1],
                            )
                        yield

                def run_all(gen):
                    for _ in gen:
                        pass

                attnT2 = attnsc.tile([P, DC, Q], pdt, tag="attnT2")

                def emit_norm(h):
                    # per-head softmax normalization, interleaved right
                    # after head h's last (q=0) PV accumulation
                    srow = rowtmp.tile([1, Q], dt.float32, tag="srow")
                    nc.vector.tensor_copy(srow[:], attn_acc[64:65, h, :])
                    rrow = rowtmp.tile([1, Q], dt.float32, tag="rrow")
                    nc.vector.reciprocal_approx_fast(rrow[:], srow[:])
                    rrow_bf = rowtmp.tile([1, Q], adt, tag="rrowbf")
                    nc.vector.tensor_copy(rrow_bf[:], rrow[:])
                    bc = lnps.tile([P, Q], dt.float32, tag="lnrow")
                    nc.tensor.matmul(
                        bc[0:64, :], ones_row_bf[0:1, 0:64], rrow_bf[:],
                        start=True, stop=True,
                    )
                    t1 = tmp.tile([HD, Q], dt.float32, tag="anorm")
                    nc.vector.tensor_tensor(
                        t1[:], attn_acc[0:64, h, :], bc[0:64, :], Alu.mult
                    )
                    off = 64 * (h // 8)
                    nc.vector.tensor_scalar_add(
                        attnT2[off:off + 64, h % 8, :], t1[:], bv_s[:, h:h + 1]
                    )

                def emit_proj(mo, h2bf):
                    wp_t = wkv.tile([P, DC, P], pdt, tag="wkq")
                    nc.sync.dma_start(wp_t[:], w_projr[mo])
                    psp = scp.tile([P, KQ], dt.float32, tag="sc")
                    for c in range(DC):
                        nc.tensor.matmul(
                            psp[:], wp_t[:, c, :], attnT2[:, c, :],
                            start=(c == 0), stop=(c == DC - 1),
                        )
                    nc.vector.scalar_tensor_tensor(
                        h2[:, mo, :], psp[:], bproj_s[:, mo:mo + 1],
                        hq_sb[:, mo, :], Alu.add, Alu.add,
                    )
                    # bf16 mirror feeds the LN2 stats matmuls + apply
                    nc.scalar.activation(h2bf[:, mo, :], h2[:, mo, :], F.Copy)

                et_t = {}

                def emit_scores_pair(q, i, NQ, qsl, pull):
                    # Heads 2i / 2i+1 live on partition halves 0:64 / 64:128
                    # of chunk i. Their K=64 score MMs auto-derive
                    # tile_position (0,0) / (64,0); issued back-to-back they
                    # run CONCURRENTLY on disjoint PE row groups (~2x).
                    he, ho = 2 * i, 2 * i + 1
                    ete = etp.tile([P, NKT, KQ], adt, tag="et")
                    eto = etp.tile([P, NKT, KQ], adt, tag="et")
                    et_t[he], et_t[ho] = ete, eto
                    if NQ <= 256:
                        # [P, 2, NQ] fits one PSUM bank: pair the score MMs
                        # and halve the exp-ACT count
                        for half in range(2):
                            pse = scp.tile([P, 2, 256], dt.float32, tag="sc")
                            pso = scp.tile([P, 2, 256], dt.float32, tag="sc")
                            for k2 in range(2):
                                kt = half * 2 + k2
                                nc.tensor.matmul(
                                    pse[:, k2, 0:NQ],
                                    kt_sb[0:64, i, kt * P:(kt + 1) * P],
                                    qt[0:64, i, qsl],
                                    start=True, stop=True,
                                )
                                nc.tensor.matmul(
                                    pso[:, k2, 0:NQ],
                                    kt_sb[64:128, i, kt * P:(kt + 1) * P],
                                    qt[64:128, i, qsl],
                                    start=True, stop=True,
                                )
                                pull()
                            for et, psc in ((ete, pse), (eto, pso)):
                                nc.scalar.activation(
                                    et[:, half * 2:half * 2 + 2, 0:NQ],
                                    psc[:, :, 0:NQ], F.Exp, scale=0.125,
                                )
                                nc.gpsimd.tensor_tensor(
                                    et[:, half * 2:half * 2 + 2, 0:P],
                                    et[:, half * 2:half * 2 + 2, 0:P],
                                    dmask_s[:, half * 2:half * 2 + 2, :],
                                    Alu.mult,
                                )
                                pull()
                            pull()
                        return
                    for kt in range(NKT):
                        pse = scp.tile([P, KQ], dt.float32, tag="sc")
                        pso = scp.tile([P, KQ], dt.float32, tag="sc")
                        nc.tensor.matmul(
                            pse[:, 0:NQ],
                            kt_sb[0:64, i, kt * P:(kt + 1) * P],
                            qt[0:64, i, qsl],
                            start=True, stop=True,
                        )
                        nc.tensor.matmul(
                            pso[:, 0:NQ],
                            kt_sb[64:128, i, kt * P:(kt + 1) * P],
                            qt[64:128, i, qsl],
                            start=True, stop=True,
                        )
                        pull()
                        nc.scalar.activation(
                            ete[:, kt, 0:NQ], pse[:, 0:NQ], F.Exp, scale=0.125,
                        )
                        nc.scalar.activation(
                            eto[:, kt, 0:NQ], pso[:, 0:NQ], F.Exp, scale=0.125,
                        )
                        if kt % 2 == 1:
                            # multiplicative causal mask on the diagonal
                            # query tile (first 128 columns of the window)
                            nc.gpsimd.tensor_tensor(
                                ete[:, kt - 1:kt + 1, 0:P],
                                ete[:, kt - 1:kt + 1, 0:P],
                                dmask_s[:, kt - 1:kt + 1, :], Alu.mult,
                            )
                            nc.gpsimd.tensor_tensor(
                                eto[:, kt - 1:kt + 1, 0:P],
                                eto[:, kt - 1:kt + 1, 0:P],
                                dmask_s[:, kt - 1:kt + 1, :], Alu.mult,
                            )
                        pull()

                def emit_pv(q, h, NQ, qsl, pull=None):
                    et = et_t.pop(h)
                    pa = pvps.tile([65, KQ], dt.float32, tag="pv")
                    for kt in range(NKT):
                        nc.tensor.matmul(
                            pa[:, qsl], v_sb[:, kt, h * 65:h * 65 + 65],
                            et[:, kt, 0:NQ],
                            start=(kt == 0), stop=(kt == NKT - 1),
                        )
                        if pull is not None:
                            pull()
                    nc.scalar.activation(
                        attn_acc[:, h, q * P:(q + 1) * P],
                        pa[:, q * P:(q + 1) * P], F.Copy,
                    )
                    if q < NQT - 1:
                        nc.vector.tensor_tensor(
                            attn_acc[:, h, (q + 1) * P:],
                            attn_acc[:, h, (q + 1) * P:],
                            pa[:, (q + 1) * P:], Alu.add,
                        )
                    if q == 0:
                        emit_norm(h)

                # prologue: quarter 3 LN fully, eagerly; own-rows DMA for
                # the residual path is issued after the critical q3 data
                load_quarter(3)
                # V weights are quarter-invariant: fetch once, keep resident
                wv0 = wvp.tile([P, DC, 512], adt, tag="wv")
                nc.sync.dma_start(wv0[:], w_v[0])
                wv1 = wvp.tile([P, DC, 512], adt, tag="wv")
                nc.sync.dma_start(wv1[:], w_v[1])
                xln3 = start_quarter(3, sq_dve=True)
                mean3, rstd3 = stats.pop(3)
                mb3, rb3 = ln_apply_start(mean3, rstd3, lnps, "lnrow")
                for c in range(DC):
                    ln_apply_chunk(xchunk(xln3), mb3, rb3, xln3, c)

                for q in range(NQT - 1, -1, -1):
                    if q == 1:
                        # residual own-rows, needed only by the proj phase
                        for c in range(DC):
                            nc.sync.dma_start(
                                hq_sb[:, c, :],
                                hT_r[:, c, :].rearrange(
                                    "p (j n) -> p j n", n=KQ
                                )[:, :, 384:512],
                            )
                    xln = xln_t.pop(q)
                    nxt = None
                    if q > 0:
                        load_quarter(q - 1)
                        xln_n = start_quarter(q - 1)
                        nxt = (*stats.pop(q - 1), xln_n)

                    # preamble: K0, V(vs0) x4, Q0..Q3 dense (scp slots)
                    run_all(g_k(xln, 0, scp, "sc"))
                    for kt in range(NKT):
                        run_all(g_v(xln, wv0, 0, kt, scp, "sc"))
                    run_all(g_k(xln, 1, scp, "sc"))
                    for p in range(4):
                        run_all(g_q(xln, q, p, scp, "sc"))

                    # fine-grained fill queue: two MMs pulled after every
                    # score so the PE never drains while exp catches up
                    def fill_iter():
                        yield from g_k(xln, 2, fillp, "mm", on_scalar=(q > 0))
                        for kt in range(2):
                            yield from g_v(xln, wv1, 1, kt, fillp, "mm")
                        yield from g_k(xln, 3, fillp, "mm", on_scalar=(q > 0))
                        for kt in range(2, 4):
                            yield from g_v(xln, wv1, 1, kt, fillp, "mm")
                        yield from g_k(xln, 4, fillp, "mm", on_scalar=(q > 0))
                        yield from g_q(xln, q, 4, fillp, "mm")
                        yield from g_k(xln, 5, fillp, "mm", on_scalar=(q > 0))
                        yield from g_q(xln, q, 5, fillp, "mm")
                        yield from g_k(xln, 6, fillp, "mm", on_scalar=(q > 0))
                        yield from g_q(xln, q, 6, fillp, "mm")
                        yield from g_k(xln, 7, fillp, "mm", on_scalar=(q > 0))
                        yield from g_q(xln, q, 7, fillp, "mm")

                    fq = fill_iter()

                    def pull():
                        next(fq, None)

                    NQ = (NQT - q) * P
                    qsl = slice(q * P, Q)
                    # pv stagger keeps at most 3 et tiles live: iteration i
                    # consumes pair i-1's odd head and pair i's even head
                    for i in range(H // 2):
                        if i == 0 and nxt is not None:
                            mb_n, rb_n = ln_apply_start(nxt[0], nxt[1],
                                                        lnps, "lnrow")
                        emit_scores_pair(q, i, NQ, qsl, pull)
                        if i > 0:
                            emit_pv(q, 2 * i - 1, NQ, qsl, pull)
                        emit_pv(q, 2 * i, NQ, qsl, pull)
                        if nxt is not None:
                            ln_apply_chunk(xchunk(nxt[2]), mb_n, rb_n,
                                           nxt[2], i)
                    run_all(fq)
                    emit_pv(q, H - 1, NQ, qsl)

                if dbg:
                    nc.sync.dma_start(dbg_qt[:], qt[:])
                    nc.sync.dma_start(dbg_acc[:], attn_acc[:])

                # h2 bf16 mirror reuses the xln slot (dead after quarter 0)
                h2bf = xlnp.tile([P, DC, Q], adt, tag="xln")
                # LN2 stats interleave into the proj loop (lagged 2 so the
                # first stats MM doesn't stall on the attention-tail scalar
                # backlog feeding the h2bf casts)
                pss2 = pvps.tile([1, KQ], dt.float32, tag="pv")
                psq2 = pvps.tile([1, KQ], dt.float32, tag="pv")

                def ln2_stats(mo):
                    sq2 = tmp.tile([P, KQ], adt, tag="sq")
                    nc.gpsimd.tensor_tensor(
                        sq2[:], h2bf[:, mo, :], h2bf[:, mo, :], Alu.mult
                    )
                    nc.tensor.matmul(pss2[:], ones_col_bf[:], h2bf[:, mo, :],
                                     start=(mo == 0), stop=(mo == DC - 1))
                    nc.tensor.matmul(psq2[:], ones_col_bf[:], sq2[:],
                                     start=(mo == 0), stop=(mo == DC - 1))

                for mo in range(DC):
                    emit_proj(mo, h2bf)
                    if mo >= 2:
                        ln2_stats(mo - 2)
                ln2_stats(DC - 2)
                ln2_stats(DC - 1)
                mean2 = rows.tile([1, KQ], dt.float32, tag="mean")
                nc.vector.tensor_scalar_mul(mean2[:], pss2[:], 1.0 / D)
                msq2 = rowtmp.tile([1, KQ], dt.float32, tag="lnrow")
                nc.vector.tensor_tensor(msq2[:], mean2[:], mean2[:], Alu.mult)
                var2 = rowtmp.tile([1, KQ], dt.float32, tag="lnrow")
                nc.vector.scalar_tensor_tensor(
                    var2[:], psq2[:], 1.0 / D, msq2[:], Alu.mult, Alu.subtract
                )
                std2 = rowtmp.tile([1, KQ], dt.float32, tag="lnrow")
                nc.scalar.activation(std2[:], var2[:], F.Sqrt, bias=eps_t[:])
                rstd2 = rows.tile([1, KQ], dt.float32, tag="rstd")
                nc.vector.reciprocal_approx_fast(rstd2[:], std2[:])
                mean2_bf = rows.tile([1, KQ], adt, tag="meanbf")
                nc.vector.tensor_copy(mean2_bf[:], mean2[:])
                rstd2_bf = rows.tile([1, KQ], adt, tag="rstdbf")
                nc.vector.tensor_copy(rstd2_bf[:], rstd2[:])
                if dbg:
                    nc.sync.dma_start(dbg_at2[:], attnT2[:])

            # ---- LN2 / fc+gelu / mlp + residual ----
            with (
                tc.tile_pool(name="mlpsc", bufs=1) as mlpsc,
                tc.tile_pool(name="wfcs", bufs=4) as wfcs,
                tc.tile_pool(name="wmlps", bufs=4) as wmlps,
                tc.tile_pool(name="psfc", bufs=2, space="PSUM") as psfc,
                tc.tile_pool(name="psm", bufs=2, space="PSUM") as psm,
                tc.tile_pool(name="lnps2", bufs=2, space="PSUM") as lnps2,
            ):
                h2c = lambda c: h2[:, c, :]
                h2n = mlpsc.tile([P, DC, Q], pdt, tag="h2n")
                ln_apply(h2c, mean2_bf, rstd2_bf, h2n, lnps2, "lnbb")
                if dbg:
                    nc.sync.dma_start(dbg_h2[:], h2[:])
                    nc.sync.dma_start(dbg_h2n[:], h2n[:])
                y2 = mlpsc.tile([P, DC, Q], dt.float32, tag="y2")
                g_half = mlpsc.tile([P, IC // 2, Q], pdt, tag="g")
                for ih in range(2):
                    for m in range(IC // 2):
                        mg = ih * (IC // 2) + m
                        wfc_t = wfcs.tile([P, DC, P], pdt, tag="wfc")
                        nc.sync.dma_start(wfc_t[:], w_fcr[mg])
                        psf = psfc.tile([P, Q], dt.float32, tag="fc")
                        for c in range(DC):
                            nc.tensor.matmul(
                                psf[:], wfc_t[:, c, :], h2n[:, c, :],
                                start=(c == 0), stop=(c == DC - 1),
                            )
                        nc.scalar.activation(
                            g_half[:, m, :], psf[:], F.Gelu,
                            bias=bfc_s[:, mg:mg + 1],
                        )
                    if dbg and ih == 0:
                        nc.sync.dma_start(dbg_g[:], g_half[:])
                    for mo in range(DC):
                        wm_t = wmlps.tile([P, IC // 2, P], pdt, tag="wmlp")
                        nc.sync.dma_start(wm_t[:], w_mlpr[ih, mo])
                        psm_ = psm.tile([P, Q], dt.float32, tag="mm2")
                        for c in range(IC // 2):
                            nc.tensor.matmul(
                                psm_[:], wm_t[:, c, :], g_half[:, c, :],
                                start=(c == 0), stop=(c == IC // 2 - 1),
                            )
                        if ih == 0:
                            nc.vector.tensor_copy(y2[:, mo, :], psm_[:])
                        else:
                            ot = tmp.tile([P, Q], dt.float32, tag="anorm")
                            nc.vector.tensor_tensor(
                                ot[:], y2[:, mo, :], psm_[:], Alu.add
                            )
                            nc.vector.scalar_tensor_tensor(
                                ot[:], ot[:], bmlp_s[:, mo:mo + 1],
                                h2[:, mo, :], Alu.add, Alu.add,
                            )
                            nc.sync.dma_start(
                                outT.rearrange("(c p) n -> p c n", p=P)[:, mo, :],
                                ot[:],
                            )

    nc.compile()
    return nc


def _get_nc(cfg):
    if cfg not in _BUILD_CACHE:
        _BUILD_CACHE[cfg] = _build(cfg)
    return _BUILD_CACHE[cfg]


def _np_dt(name):
    if name == "bf16":
        import ml_dtypes
        return ml_dtypes.bfloat16
    if name == "f16":
        return np.float16
    return np.float32


def _perm_for_core(a):
    """Key order per quarter: non-own keys in natural order, own 128 last."""
    perm = []
    for q in range(NQT):
        base = q * KQ
        own = np.arange(base + 128 * a, base + 128 * a + P)
        others = np.setdiff1d(np.arange(base, base + KQ), own)
        perm.append(np.concatenate([others, own]))
    return np.concatenate(perm)


def _prep_in_maps(inputs, cfg):
    adt_np, pdt_np = _np_dt(cfg[0]), _np_dt(cfg[1])
    h = np.asarray(inputs["hidden_states"], dtype=np.float32)
    w_qkv = np.asarray(inputs["w_qkv"], np.float32)
    b_qkv = np.asarray(inputs["b_qkv"], np.float32)
    g1 = np.asarray(inputs["g1"], np.float32)
    be1 = np.asarray(inputs["be1"], np.float32)
    g2 = np.asarray(inputs["g2"], np.float32)
    be2 = np.asarray(inputs["be2"], np.float32)

    # fold LN1 gamma/beta into the qkv weights and biases
    w_qkv_f = w_qkv * g1[:, None]
    b_qkv_f = b_qkv + be1 @ w_qkv
    w_fc = np.asarray(inputs["w_fc"], np.float32)
    b_fc = np.asarray(inputs["b_fc"], np.float32)
    w_fc_f = w_fc * g2[:, None]
    b_fc_f = b_fc + be2 @ w_fc

    def chunk_w(w, p=P):  # [Din, N] -> [p, Din//p, N]
        return np.ascontiguousarray(w.reshape(-1, p, w.shape[1]).transpose(1, 0, 2))

    def pvec(v, p=P):  # [n*p] -> [p, n]
        return np.ascontiguousarray(v.reshape(-1, p).T)

    def mslice(a, nsl):  # [p, c, n] -> [n//nsl, p, c, nsl]
        p, c, n = a.shape
        return np.ascontiguousarray(
            a.reshape(p, c, n // nsl, nsl).transpose(2, 0, 1, 3)
        )

    wq = mslice(chunk_w(w_qkv_f[:, 0:D]), P)
    wk = mslice(chunk_w(w_qkv_f[:, D:2 * D]), P)
    wv = mslice(chunk_w(w_qkv_f[:, 2 * D:3 * D]), 512)

    # proj with head pairs (c, c+8) stacked on the 128 contraction rows
    w_proj = np.asarray(inputs["w_proj"], np.float32)
    w2 = w_proj.reshape(H, HD, D)
    wp_t = np.concatenate([w2[0:8], w2[8:16]], axis=1)  # [8, 128, 1024]
    wp = np.ascontiguousarray(
        wp_t.reshape(DC, P, DC, P).transpose(2, 1, 0, 3)
    )

    wfc = mslice(chunk_w(w_fc_f), P)
    wm = chunk_w(np.asarray(inputs["w_mlp"], np.float32))  # [128, 32, 1024]
    wmlp = np.ascontiguousarray(
        wm.reshape(P, 2, IC // 2, DC, P).transpose(1, 3, 0, 2, 4)
    )

    shared = {
        "w_q": wq.astype(adt_np), "w_k": wk.astype(adt_np),
        "w_v": wv.astype(adt_np), "w_projr": wp.astype(pdt_np),
        "w_fcr": wfc.astype(pdt_np), "w_mlpr": wmlp.astype(pdt_np),
        "bq": pvec(b_qkv_f[0:D]),
        "bk": pvec(b_qkv_f[D:2 * D]),
        "bv": pvec(b_qkv_f[2 * D:3 * D], p=HD),
        "bproj": pvec(np.asarray(inputs["b_proj"], np.float32)),
        "bfc": pvec(b_fc_f),
        "bmlp": pvec(np.asarray(inputs["b_mlp"], np.float32)),
    }
    in_maps = []
    for core in range(8):
        b, a = core // 4, core % 4
        perm = _perm_for_core(a)
        hrot = h[b, perm]
        # multiplicative 0/1 mask on exp'd scores for the diagonal query
        # tile: key position j=kt*128+p in the quarter vs query row r.
        pp = np.arange(P)
        dm = np.zeros((P, NKT, P), np.float32)
        for kt in range(NKT - 1):
            dm[:, kt, :] = ((kt * P + pp) < (P * a)).astype(np.float32)[:, None]
        dm[:, NKT - 1, :] = (pp[:, None] <= pp[None, :]).astype(np.float32)
        in_maps.append(
            dict(
                shared,
                hT=np.ascontiguousarray(hrot.T),
                dmask=dm.astype(adt_np),
            )
        )
    return in_maps


def _stitch(results):
    out = np.empty((2, S, D), dtype=np.float32)
    for core in range(8):
        b, a = core // 4, core % 4
        r = results[core]["outT"].T  # [512, D]: cols j*128+p -> row 512j+128a+p
        for j in range(NQT):
            out[b, j * KQ + P * a: j * KQ + P * a + P] = r[j * P:(j + 1) * P]
    return out


def run(inputs, cfg=("bf16", "bf16"), trace=False, trace_cores=None):
    nc = _get_nc(cfg)
    in_maps = _prep_in_maps(inputs, cfg)
    res = bass_utils.run_bass_kernel_spmd(
        nc, in_maps, core_ids=list(range(8)), trace=trace, trace_cores=trace_cores
    )
    return _stitch(res.results), res


def kernel(**inputs) -> np.ndarray:
    out, _ = run(inputs, cfg=("bf16", "bf16"))
    return out



# revision 73
# speedup vs baseline: 1.4935x; 1.0073x over previous
"""GPT2 block kernel for 8 TRN2 NeuronCores (Bass/Tile, SPMD).

Sharding: the 4096 rows (batch*seq) are split 8 ways -> 512 rows/core
(4 cores per batch element). Core (b, a) owns query blocks {4j + a}
(128 rows each, one per key-quarter j) of batch b. Each core
redundantly computes K,V for its batch, but scores/PV/exp only for
(quarter q) x (own query tiles j >= q) -- 62.5% of the full rectangle.
Quarters are processed in DESCENDING order so query tile j (produced
from quarter j's LayerNorm output) exists before quarters q < j consume
it. Zero collectives.

Per-core key permutation (host side): within each quarter, the core's
own 128 rows are moved to the last 128 key positions, so the Q-proj
input is always xln[:, :, 384:512] -- uniform addresses across cores;
the causal mask becomes a per-core data tensor (multiplicative 0/1 on
the exp'd scores, applied by the otherwise-idle GpSimd engine).

LN gamma/beta are folded into the consuming weights host-side, exp is
batched into [128, 2, N] activations, reciprocals use the fast approx
DVE op, and proj packs head pairs to a full K=128 contraction.
"""

import numpy as np
import sys

sys.path.insert(0, "/opt/trn_rl_repo")

import concourse.bacc as bacc
import concourse.mybir as mybir
import concourse.tile as tile
from concourse import bass_utils

dt = mybir.dt
F = mybir.ActivationFunctionType
Alu = mybir.AluOpType

D = 1024
S = 2048
Q = 512        # own rows per core
H = 16
HD = 64
INNER = 4096
P = 128
DC = D // P    # 8
IC = INNER // P  # 32
EPS = 1e-5
NQT = 4        # key quarters
KQ = S // NQT  # 512 keys per quarter
NKT = KQ // P  # 4 key tiles of 128 per quarter

_BUILD_CACHE = {}

_DT = {"f32": dt.float32, "f32r": dt.float32r, "bf16": dt.bfloat16, "f16": dt.float16}


def _build(cfg, dbg=False):
    adt = _DT[cfg[0]]   # attention path: qkv/scores/PV operands
    pdt = _DT[cfg[1]]   # proj/fc/mlp path operands
    nc = bacc.Bacc("TRN2", target_bir_lowering=False, debug=False)
    if dbg:
        dbg_xln = nc.dram_tensor("dbg_xln", [NQT, P, DC, KQ], adt,
                                 kind="ExternalOutput")
        dbg_qt = nc.dram_tensor("dbg_qt", [P, DC, Q], adt,
                                kind="ExternalOutput")
        dbg_kt = nc.dram_tensor("dbg_kt", [NQT, P, DC, KQ], adt,
                                kind="ExternalOutput")
        dbg_acc = nc.dram_tensor("dbg_acc", [65, H, Q], dt.float32,
                                 kind="ExternalOutput")
        dbg_et = nc.dram_tensor("dbg_et", [NQT, P, NKT, KQ], adt,
                                kind="ExternalOutput")
        dbg_at2 = nc.dram_tensor("dbg_at2", [P, DC, Q], pdt,
                                 kind="ExternalOutput")
        dbg_h2 = nc.dram_tensor("dbg_h2", [P, DC, Q], dt.float32,
                                kind="ExternalOutput")
        dbg_h2n = nc.dram_tensor("dbg_h2n", [P, DC, Q], pdt,
                                 kind="ExternalOutput")
        dbg_g = nc.dram_tensor("dbg_g", [P, IC // 2, Q], pdt,
                               kind="ExternalOutput")

    hT = nc.dram_tensor("hT", [D, S], dt.float32, kind="ExternalInput")
    dmask = nc.dram_tensor("dmask", [P, NKT, P], adt, kind="ExternalInput")
    # weights arrive pre-tiled from the host in consumption order
    w_q = nc.dram_tensor("w_q", [DC, P, DC, P], adt, kind="ExternalInput")
    w_k = nc.dram_tensor("w_k", [DC, P, DC, P], adt, kind="ExternalInput")
    w_v = nc.dram_tensor("w_v", [2, P, DC, 512], adt, kind="ExternalInput")
    w_projr = nc.dram_tensor("w_projr", [DC, P, DC, P], pdt, kind="ExternalInput")
    w_fcr = nc.dram_tensor("w_fcr", [IC, P, DC, P], pdt, kind="ExternalInput")
    w_mlpr = nc.dram_tensor("w_mlpr", [2, DC, P, IC // 2, P], pdt, kind="ExternalInput")
    bq = nc.dram_tensor("bq", [P, DC], dt.float32, kind="ExternalInput")
    bk = nc.dram_tensor("bk", [P, DC], dt.float32, kind="ExternalInput")
    bv = nc.dram_tensor("bv", [HD, H], dt.float32, kind="ExternalInput")
    bproj = nc.dram_tensor("bproj", [P, DC], dt.float32, kind="ExternalInput")
    bfc = nc.dram_tensor("bfc", [P, IC], dt.float32, kind="ExternalInput")
    bmlp = nc.dram_tensor("bmlp", [P, DC], dt.float32, kind="ExternalInput")
    outT = nc.dram_tensor("outT", [D, Q], dt.float32, kind="ExternalOutput")

    hT_r = hT.rearrange("(c p) n -> p c n", p=P)

    with tile.TileContext(nc) as tc:
        with (
            tc.tile_pool(name="const", bufs=1) as const,
            tc.tile_pool(name="rows", bufs=2) as rows,
            tc.tile_pool(name="tmp", bufs=2) as tmp,
            tc.tile_pool(name="rowtmp", bufs=2) as rowtmp,
            tc.tile_pool(name="persist", bufs=1) as persist,
        ):
            ones_col = const.tile([P, 1], dt.float32)
            nc.vector.memset(ones_col[:], 1.0)
            ones_row = const.tile([1, P], dt.float32)
            nc.vector.memset(ones_row[:], 1.0)
            # bf16 ones: fp32 matmuls run LOW_HIGH two-pass at 4 cycles/row,
            # so LN stats / row-broadcast matmuls use bf16 operands instead
            ones_col_bf = const.tile([P, 1], adt)
            nc.vector.memset(ones_col_bf[:], 1.0)
            ones_row_bf = const.tile([1, P], adt)
            nc.vector.memset(ones_row_bf[:], 1.0)
            eps_t = const.tile([1, 1], dt.float32)
            nc.vector.memset(eps_t[:], EPS)

            def load_pvec(t):
                s = const.tile(list(t.shape), dt.float32, tag=t.name)
                nc.sync.dma_start(s[:], t[:])
                return s

            bq_s, bk_s, bv_s = load_pvec(bq), load_pvec(bk), load_pvec(bv)
            bproj_s, bfc_s, bmlp_s = load_pvec(bproj), load_pvec(bfc), load_pvec(bmlp)

            h2 = persist.tile([P, DC, Q], dt.float32, tag="h2")
            hq_sb = persist.tile([P, DC, Q], dt.float32, tag="hq")

            # LN stats for a [P, DC, KQ] fp32 block resident in SBUF.
            # sq on GpSimd, partition-sums via PE ones-matmuls, row math on
            # DVE, sqrt on scalar, reciprocal via fast DVE approx.
            def ln_stats(get_chunk, lnps, tag, rowtag="lnrow", sq_dve=False):
                # Each chunk is cast to a rotating bf16 scratch tile consumed
                # immediately by the two ones-matmuls (bf16 = 1 cycle/row on
                # the PE vs 4 for fp32); the apply pass still reads the fp32
                # source, so nothing is rewritten in place. Per-element bf16
                # rounding averages out over the 1024-wide sums.
                pss = lnps.tile([1, KQ], dt.float32, tag=rowtag)
                psq = lnps.tile([1, KQ], dt.float32, tag=rowtag)
                eng = nc.vector if sq_dve else nc.gpsimd
                for c in range(DC):
                    xc = get_chunk(c)
                    xraw = tmp.tile([P, KQ], adt, tag="xraw")
                    nc.scalar.activation(xraw[:], xc, F.Copy)
                    sq = tmp.tile([P, KQ], adt, tag="sq")
                    eng.tensor_tensor(sq[:], xraw[:], xraw[:], Alu.mult)
                    nc.tensor.matmul(pss[:], ones_col_bf[:], xraw[:],
                                     start=(c == 0), stop=(c == DC - 1))
                    nc.tensor.matmul(psq[:], ones_col_bf[:], sq[:],
                                     start=(c == 0), stop=(c == DC - 1))
                mean = rows.tile([1, KQ], dt.float32, tag="mean", bufs=1)
                nc.vector.tensor_scalar_mul(mean[:], pss[:], 1.0 / D)
                msq = rowtmp.tile([1, KQ], dt.float32, tag="lnrow")
                nc.vector.tensor_tensor(msq[:], mean[:], mean[:], Alu.mult)
                var = rowtmp.tile([1, KQ], dt.float32, tag="lnrow")
                nc.vector.scalar_tensor_tensor(
                    var[:], psq[:], 1.0 / D, msq[:], Alu.mult, Alu.subtract
                )
                std = rowtmp.tile([1, KQ], dt.float32, tag="lnrow")
                nc.scalar.activation(std[:], var[:], F.Sqrt, bias=eps_t[:])
                rstd = rows.tile([1, KQ], dt.float32, tag="rstd", bufs=1)
                nc.vector.reciprocal_approx_fast(rstd[:], std[:])
                # bf16 row copies feed the bf16 broadcast matmuls
                mean_bf = rows.tile([1, KQ], adt, tag="meanbf")
                nc.vector.tensor_copy(mean_bf[:], mean[:])
                rstd_bf = rows.tile([1, KQ], adt, tag="rstdbf")
                nc.vector.tensor_copy(rstd_bf[:], rstd[:])
                return mean_bf, rstd_bf

            # apply pass: xln = (x - mean) * rstd (gamma/beta folded into
            # the consuming weights host-side), written as `odt`.
            def ln_apply_start(mean, rstd, bbpool, bbtag):
                mb = bbpool.tile([P, KQ], dt.float32, tag=bbtag)
                rb = bbpool.tile([P, KQ], dt.float32, tag=bbtag)
                nc.tensor.matmul(mb[:], ones_row_bf[:], mean[:],
                                 start=True, stop=True)
                nc.tensor.matmul(rb[:], ones_row_bf[:], rstd[:],
                                 start=True, stop=True)
                return mb, rb

            def ln_apply_chunk(get_chunk, mb, rb, out, c, gp=False):
                eng = nc.gpsimd if gp else nc.vector
                xc = get_chunk(c)
                t1 = tmp.tile([P, KQ], adt, tag="lnt1")
                eng.tensor_tensor(t1[:], xc, mb[:], Alu.subtract)
                eng.tensor_tensor(out[:, c, :], t1[:], rb[:], Alu.mult)

            def ln_apply(get_chunk, mean, rstd, out, bbpool, bbtag,
                         gp_chunks=0):
                mb, rb = ln_apply_start(mean, rstd, bbpool, bbtag)
                for c in range(DC):
                    ln_apply_chunk(get_chunk, mb, rb, out, c,
                                   gp=(c >= DC - gp_chunks))

            with (
                tc.tile_pool(name="attnsc", bufs=1) as attnsc,
                tc.tile_pool(name="hqp", bufs=2) as hqp,
                tc.tile_pool(name="xlnp", bufs=1) as xlnp,
                tc.tile_pool(name="wkv", bufs=4) as wkv,
                tc.tile_pool(name="wvp", bufs=2) as wvp,
                tc.tile_pool(name="etp", bufs=3) as etp,
                tc.tile_pool(name="scp", bufs=3, space="PSUM") as scp,
                tc.tile_pool(name="fillp", bufs=1, space="PSUM") as fillp,
                tc.tile_pool(name="pvps", bufs=2, space="PSUM") as pvps,
                tc.tile_pool(name="lnps", bufs=2, space="PSUM") as lnps,
            ):
                qt = attnsc.tile([P, DC, Q], adt, tag="qt")
                attn_acc = attnsc.tile([65, H, Q], dt.float32, tag="attn_acc")
                kt_sb = attnsc.tile([P, DC, KQ], adt, tag="kt")
                v_sb = attnsc.tile([P, NKT, H * 65], adt, tag="v")
                vview = v_sb[:].rearrange("p k (h x) -> p k h x", x=65)
                nc.vector.tensor_copy(
                    vview[:, :, :, 64:65],
                    ones_col[:].to_broadcast([P, NKT, H, 1]),
                )
                dmask_s = const.tile([P, NKT, P], adt, tag="dmask")
                nc.sync.dma_start(dmask_s[:], dmask[:])

                hquart = {}

                def load_quarter(q):
                    t = hqp.tile([P, DC, KQ], dt.float32, tag="hquart")
                    for c in range(DC):
                        nc.sync.dma_start(
                            t[:, c, :], hT_r[:, c, q * KQ:(q + 1) * KQ]
                        )
                    hquart[q] = t

                def hchunk(q):
                    return lambda c: hquart[q][:, c, :]

                stats = {}

                xln_t = {}

                def make_xln_start(qq):
                    xln = xlnp.tile([P, DC, KQ], adt, tag="xln")
                    xln_t[qq] = xln
                    mean, rstd = stats.pop(qq)
                    return mean, rstd, xln

                def g_k(xln, p, pool, tag, on_scalar=False):
                    wk_t = wkv.tile([P, DC, P], adt, tag="wkq")
                    nc.sync.dma_start(wk_t[:], w_k[p])
                    psk = pool.tile([P, KQ], dt.float32, tag=tag)
                    for c in range(DC):
                        nc.tensor.matmul(
                            psk[:], wk_t[:, c, :], xln[:, c, :],
                            start=(c == 0), stop=(c == DC - 1),
                        )
                        if c == DC - 1:
                            if on_scalar:
                                nc.scalar.activation(
                                    kt_sb[:, p, :], psk[:], F.Identity,
                                    bias=bk_s[:, p:p + 1],
                                )
                            else:
                                nc.vector.tensor_scalar_add(
                                    kt_sb[:, p, :], psk[:], bk_s[:, p:p + 1]
                                )
                        yield

                def g_v(xln, wv_t, vs, kt, pool, tag):
                    psv = pool.tile([P, KQ], dt.float32, tag=tag)
                    for c in range(DC):
                        nc.tensor.matmul(
                            psv[:], xln[:, c, kt * P:(kt + 1) * P],
                            wv_t[:, c, :],
                            start=(c == 0), stop=(c == DC - 1),
                        )
                        if c == DC - 1:
                            dst = v_sb[
                                :, kt, vs * 8 * 65:(vs + 1) * 8 * 65
                            ].rearrange("p (h x) -> p h x", x=65)[:, :, 0:64]
                            nc.vector.tensor_copy(
                                dst, psv[:].rearrange("p (h x) -> p h x", x=64)
                            )
                        yield

                def g_q(xln, q, p, pool, tag):
                    wq_t = wkv.tile([P, DC, P], adt, tag="wkq")
                    nc.sync.dma_start(wq_t[:], w_q[p])
                    psq_ = pool.tile([P, KQ], dt.float32, tag=tag)
                    for c in range(DC):
                        nc.tensor.matmul(
                            psq_[:, 0:P], wq_t[:, c, :], xln[:, c, 384:512],
                            start=(c == 0), stop=(c == DC - 1),
                        )
                        if c == DC - 1:
                            nc.vector.tensor_scalar_add(
                                qt[:, p, q * P:(q + 1) * P], psq_[:, 0:P],
                                bq_s[:, p:p + 1],
                            )
                        yield

                def run_all(gen):
                    for _ in gen:
                        pass

                attnT2 = attnsc.tile([P, DC, Q], pdt, tag="attnT2")

                def emit_norm(h):
                    # per-head softmax normalization, interleaved right
                    # after head h's last (q=0) PV accumulation
                    srow = rowtmp.tile([1, Q], dt.float32, tag="srow")
                    nc.vector.tensor_copy(srow[:], attn_acc[64:65, h, :])
                    rrow = rowtmp.tile([1, Q], dt.float32, tag="rrow")
                    nc.vector.reciprocal_approx_fast(rrow[:], srow[:])
                    rrow_bf = rowtmp.tile([1, Q], adt, tag="rrowbf")
                    nc.vector.tensor_copy(rrow_bf[:], rrow[:])
                    bc = lnps.tile([P, Q], dt.float32, tag="lnrow")
                    nc.tensor.matmul(
                        bc[0:64, :], ones_row_bf[0:1, 0:64], rrow_bf[:],
                        start=True, stop=True,
                    )
                    t1 = tmp.tile([HD, Q], dt.float32, tag="anorm")
                    nc.vector.tensor_tensor(
                        t1[:], attn_acc[0:64, h, :], bc[0:64, :], Alu.mult
                    )
                    off = 64 * (h // 8)
                    nc.vector.tensor_scalar_add(
                        attnT2[off:off + 64, h % 8, :], t1[:], bv_s[:, h:h + 1]
                    )

                def emit_proj(mo):
                    wp_t = wkv.tile([P, DC, P], pdt, tag="wkq")
                    nc.sync.dma_start(wp_t[:], w_projr[mo])
                    psp = scp.tile([P, KQ], dt.float32, tag="sc")
                    for c in range(DC):
                        nc.tensor.matmul(
                            psp[:], wp_t[:, c, :], attnT2[:, c, :],
                            start=(c == 0), stop=(c == DC - 1),
                        )
                    nc.vector.scalar_tensor_tensor(
                        h2[:, mo, :], psp[:], bproj_s[:, mo:mo + 1],
                        hq_sb[:, mo, :], Alu.add, Alu.add,
                    )

                et_t = {}

                def emit_scores_pair(q, i, NQ, qsl, pull):
                    # Heads 2i / 2i+1 live on partition halves 0:64 / 64:128
                    # of chunk i. Their K=64 score MMs auto-derive
                    # tile_position (0,0) / (64,0); issued back-to-back they
                    # run CONCURRENTLY on disjoint PE row groups (~2x).
                    he, ho = 2 * i, 2 * i + 1
                    ete = etp.tile([P, NKT, KQ], adt, tag="et")
                    eto = etp.tile([P, NKT, KQ], adt, tag="et")
                    et_t[he], et_t[ho] = ete, eto
                    if NQ <= 256:
                        # [P, 2, NQ] fits one PSUM bank: pair the score MMs
                        # and halve the exp-ACT count
                        for half in range(2):
                            pse = scp.tile([P, 2, 256], dt.float32, tag="sc")
                            pso = scp.tile([P, 2, 256], dt.float32, tag="sc")
                            for k2 in range(2):
                                kt = half * 2 + k2
                                nc.tensor.matmul(
                                    pse[:, k2, 0:NQ],
                                    kt_sb[0:64, i, kt * P:(kt + 1) * P],
                                    qt[0:64, i, qsl],
                                    start=True, stop=True,
                                )
                                nc.tensor.matmul(
                                    pso[:, k2, 0:NQ],
                                    kt_sb[64:128, i, kt * P:(kt + 1) * P],
                                    qt[64:128, i, qsl],
                                    start=True, stop=True,
                                )
                                pull()
                            for et, psc in ((ete, pse), (eto, pso)):
                                nc.scalar.activation(
                                    et[:, half * 2:half * 2 + 2, 0:NQ],
                                    psc[:, :, 0:NQ], F.Exp, scale=0.125,
                                )
                                nc.gpsimd.tensor_tensor(
                                    et[:, half * 2:half * 2 + 2, 0:P],
                                    et[:, half * 2:half * 2 + 2, 0:P],
                                    dmask_s[:, half * 2:half * 2 + 2, :],
                                    Alu.mult,
                                )
                                pull()
                            pull()
                        return
                    for kt in range(NKT):
                        pse = scp.tile([P, KQ], dt.float32, tag="sc")
                        pso = scp.tile([P, KQ], dt.float32, tag="sc")
                        nc.tensor.matmul(
                            pse[:, 0:NQ],
                            kt_sb[0:64, i, kt * P:(kt + 1) * P],
                            qt[0:64, i, qsl],
                            start=True, stop=True,
                        )
                        nc.tensor.matmul(
                            pso[:, 0:NQ],
                            kt_sb[64:128, i, kt * P:(kt + 1) * P],
                            qt[64:128, i, qsl],
                            start=True, stop=True,
                        )
                        pull()
                        nc.scalar.activation(
                            ete[:, kt, 0:NQ], pse[:, 0:NQ], F.Exp, scale=0.125,
                        )
                        nc.scalar.activation(
                            eto[:, kt, 0:NQ], pso[:, 0:NQ], F.Exp, scale=0.125,
                        )
                        if kt % 2 == 1:
                            # multiplicative causal mask on the diagonal
                            # query tile (first 128 columns of the window)
                            nc.gpsimd.tensor_tensor(
                                ete[:, kt - 1:kt + 1, 0:P],
                                ete[:, kt - 1:kt + 1, 0:P],
                                dmask_s[:, kt - 1:kt + 1, :], Alu.mult,
                            )
                            nc.gpsimd.tensor_tensor(
                                eto[:, kt - 1:kt + 1, 0:P],
                                eto[:, kt - 1:kt + 1, 0:P],
                                dmask_s[:, kt - 1:kt + 1, :], Alu.mult,
                            )
                        pull()

                def emit_pv(q, h, NQ, qsl, pull=None):
                    et = et_t.pop(h)
                    pa = pvps.tile([65, KQ], dt.float32, tag="pv")
                    for kt in range(NKT):
                        nc.tensor.matmul(
                            pa[:, qsl], v_sb[:, kt, h * 65:h * 65 + 65],
                            et[:, kt, 0:NQ],
                            start=(kt == 0), stop=(kt == NKT - 1),
                        )
                        if pull is not None:
                            pull()
                    nc.scalar.activation(
                        attn_acc[:, h, q * P:(q + 1) * P],
                        pa[:, q * P:(q + 1) * P], F.Copy,
                    )
                    if q < NQT - 1:
                        nc.vector.tensor_tensor(
                            attn_acc[:, h, (q + 1) * P:],
                            attn_acc[:, h, (q + 1) * P:],
                            pa[:, (q + 1) * P:], Alu.add,
                        )
                    if q == 0:
                        emit_norm(h)

                # prologue: quarter 3 LN fully, eagerly; own-rows DMA for
                # the residual path is issued after the critical q3 data
                load_quarter(3)
                # V weights are quarter-invariant: fetch once, keep resident
                wv0 = wvp.tile([P, DC, 512], adt, tag="wv")
                nc.sync.dma_start(wv0[:], w_v[0])
                wv1 = wvp.tile([P, DC, 512], adt, tag="wv")
                nc.sync.dma_start(wv1[:], w_v[1])
                stats[3] = ln_stats(hchunk(3), pvps, "3", rowtag="pv", sq_dve=True)
                mean3, rstd3, xln3 = make_xln_start(3)
                mb3, rb3 = ln_apply_start(mean3, rstd3, lnps, "lnrow")
                for c in range(DC):
                    ln_apply_chunk(hchunk(3), mb3, rb3, xln3, c)

                for q in range(NQT - 1, -1, -1):
                    if q == 1:
                        # residual own-rows, needed only by the proj phase
                        for c in range(DC):
                            nc.sync.dma_start(
                                hq_sb[:, c, :],
                                hT_r[:, c, :].rearrange(
                                    "p (j n) -> p j n", n=KQ
                                )[:, :, 384:512],
                            )
                    xln = xln_t.pop(q)
                    nxt = None
                    if q > 0:
                        load_quarter(q - 1)
                        stats[q - 1] = ln_stats(hchunk(q - 1), pvps,
                                                str(q - 1), rowtag="pv")
                        nxt = make_xln_start(q - 1)

                    # preamble: K0, V(vs0) x4, Q0..Q3 dense (scp slots)
                    run_all(g_k(xln, 0, scp, "sc"))
                    for kt in range(NKT):
                        run_all(g_v(xln, wv0, 0, kt, scp, "sc"))
                    run_all(g_k(xln, 1, scp, "sc"))
                    for p in range(4):
                        run_all(g_q(xln, q, p, scp, "sc"))

                    # fine-grained fill queue: two MMs pulled after every
                    # score so the PE never drains while exp catches up
                    def fill_iter():
                        yield from g_k(xln, 2, fillp, "mm", on_scalar=(q > 0))
                        for kt in range(2):
                            yield from g_v(xln, wv1, 1, kt, fillp, "mm")
                        yield from g_k(xln, 3, fillp, "mm", on_scalar=(q > 0))
                        for kt in range(2, 4):
                            yield from g_v(xln, wv1, 1, kt, fillp, "mm")
                        yield from g_k(xln, 4, fillp, "mm", on_scalar=(q > 0))
                        yield from g_q(xln, q, 4, fillp, "mm")
                        yield from g_k(xln, 5, fillp, "mm", on_scalar=(q > 0))
                        yield from g_q(xln, q, 5, fillp, "mm")
                        yield from g_k(xln, 6, fillp, "mm", on_scalar=(q > 0))
                        yield from g_q(xln, q, 6, fillp, "mm")
                        yield from g_k(xln, 7, fillp, "mm", on_scalar=(q > 0))
                        yield from g_q(xln, q, 7, fillp, "mm")

                    fq = fill_iter()

                    def pull():
                        next(fq, None)

                    NQ = (NQT - q) * P
                    qsl = slice(q * P, Q)
                    # pv stagger keeps at most 3 et tiles live: iteration i
                    # consumes pair i-1's odd head and pair i's even head
                    for i in range(H // 2):
                        if i == 0 and nxt is not None:
                            mb_n, rb_n = ln_apply_start(nxt[0], nxt[1],
                                                        lnps, "lnrow")
                        emit_scores_pair(q, i, NQ, qsl, pull)
                        if i > 0:
                            emit_pv(q, 2 * i - 1, NQ, qsl, pull)
                        emit_pv(q, 2 * i, NQ, qsl, pull)
                        if nxt is not None:
                            ln_apply_chunk(hchunk(q - 1), mb_n, rb_n,
                                           nxt[2], i)
                    run_all(fq)
                    emit_pv(q, H - 1, NQ, qsl)

                if dbg:
                    nc.sync.dma_start(dbg_qt[:], qt[:])
                    nc.sync.dma_start(dbg_acc[:], attn_acc[:])

                for mo in range(DC):
                    emit_proj(mo)

                # LN2 stats (h2 complete after proj), bf16 stats operands via
                # the same rotating-cast pattern as LN1
                pss2 = pvps.tile([1, KQ], dt.float32, tag="pv")
                psq2 = pvps.tile([1, KQ], dt.float32, tag="pv")
                for mo in range(DC):
                    h2raw = tmp.tile([P, KQ], adt, tag="xraw")
                    nc.scalar.activation(h2raw[:], h2[:, mo, :], F.Copy)
                    sq2 = tmp.tile([P, KQ], adt, tag="sq")
                    nc.gpsimd.tensor_tensor(
                        sq2[:], h2raw[:], h2raw[:], Alu.mult
                    )
                    nc.tensor.matmul(pss2[:], ones_col_bf[:], h2raw[:],
                                     start=(mo == 0), stop=(mo == DC - 1))
                    nc.tensor.matmul(psq2[:], ones_col_bf[:], sq2[:],
                                     start=(mo == 0), stop=(mo == DC - 1))
                mean2 = rows.tile([1, KQ], dt.float32, tag="mean", bufs=1)
                nc.vector.tensor_scalar_mul(mean2[:], pss2[:], 1.0 / D)
                msq2 = rowtmp.tile([1, KQ], dt.float32, tag="lnrow")
                nc.vector.tensor_tensor(msq2[:], mean2[:], mean2[:], Alu.mult)
                var2 = rowtmp.tile([1, KQ], dt.float32, tag="lnrow")
                nc.vector.scalar_tensor_tensor(
                    var2[:], psq2[:], 1.0 / D, msq2[:], Alu.mult, Alu.subtract
                )
                std2 = rowtmp.tile([1, KQ], dt.float32, tag="lnrow")
                nc.scalar.activation(std2[:], var2[:], F.Sqrt, bias=eps_t[:])
                rstd2 = rows.tile([1, KQ], dt.float32, tag="rstd", bufs=1)
                nc.vector.reciprocal_approx_fast(rstd2[:], std2[:])
                mean2_bf = rows.tile([1, KQ], adt, tag="meanbf")
                nc.vector.tensor_copy(mean2_bf[:], mean2[:])
                rstd2_bf = rows.tile([1, KQ], adt, tag="rstdbf")
                nc.vector.tensor_copy(rstd2_bf[:], rstd2[:])
                if dbg:
                    nc.sync.dma_start(dbg_at2[:], attnT2[:])

            # ---- LN2 / fc+gelu / mlp + residual ----
            with (
                tc.tile_pool(name="mlpsc", bufs=1) as mlpsc,
                tc.tile_pool(name="wfcs", bufs=4) as wfcs,
                tc.tile_pool(name="wmlps", bufs=4) as wmlps,
                tc.tile_pool(name="psfc", bufs=2, space="PSUM") as psfc,
                tc.tile_pool(name="psm", bufs=2, space="PSUM") as psm,
                tc.tile_pool(name="lnps2", bufs=2, space="PSUM") as lnps2,
            ):
                h2c = lambda c: h2[:, c, :]
                h2n = mlpsc.tile([P, DC, Q], pdt, tag="h2n")
                ln_apply(h2c, mean2_bf, rstd2_bf, h2n, lnps2, "lnbb")
                if dbg:
                    nc.sync.dma_start(dbg_h2[:], h2[:])
                    nc.sync.dma_start(dbg_h2n[:], h2n[:])
                y2 = mlpsc.tile([P, DC, Q], dt.float32, tag="y2")
                g_half = mlpsc.tile([P, IC // 2, Q], pdt, tag="g")
                for ih in range(2):
                    for m in range(IC // 2):
                        mg = ih * (IC // 2) + m
                        wfc_t = wfcs.tile([P, DC, P], pdt, tag="wfc")
                        nc.sync.dma_start(wfc_t[:], w_fcr[mg])
                        psf = psfc.tile([P, Q], dt.float32, tag="fc")
                        for c in range(DC):
                            nc.tensor.matmul(
                                psf[:], wfc_t[:, c, :], h2n[:, c, :],
                                start=(c == 0), stop=(c == DC - 1),
                            )
                        nc.scalar.activation(
                            g_half[:, m, :], psf[:], F.Gelu,
                            bias=bfc_s[:, mg:mg + 1],
                        )
                    if dbg and ih == 0:
                        nc.sync.dma_start(dbg_g[:], g_half[:])
                    for mo in range(DC):
                        wm_t = wmlps.tile([P, IC // 2, P], pdt, tag="wmlp")
                        nc.sync.dma_start(wm_t[:], w_mlpr[ih, mo])
                        psm_ = psm.tile([P, Q], dt.float32, tag="mm2")
                        for c in range(IC // 2):
                            nc.tensor.matmul(
                                psm_[:], wm_t[:, c, :], g_half[:, c, :],
                                start=(c == 0), stop=(c == IC // 2 - 1),
                            )
                        if ih == 0:
                            nc.vector.tensor_copy(y2[:, mo, :], psm_[:])
                        else:
                            ot = tmp.tile([P, Q], dt.float32, tag="anorm")
                            nc.vector.tensor_tensor(
                                ot[:], y2[:, mo, :], psm_[:], Alu.add
                            )
                            nc.vector.scalar_tensor_tensor(
                                ot[:], ot[:], bmlp_s[:, mo:mo + 1],
                                h2[:, mo, :], Alu.add, Alu.add,
                            )
                            nc.sync.dma_start(
                                outT.rearrange("(c p) n -> p c n", p=P)[:, mo, :],
                                ot[:],
                            )

    nc.compile()
    return nc


def _get_nc(cfg):
    if cfg not in _BUILD_CACHE:
        _BUILD_CACHE[cfg] = _build(cfg)
    return _BUILD_CACHE[cfg]


def _np_dt(name):
    if name == "bf16":
        import ml_dtypes
        return ml_dtypes.bfloat16
    if name == "f16":
        return np.float16
    return np.float32


def _perm_for_core(a):
    """Key order per quarter: non-own keys in natural order, own 128 last."""
    perm = []
    for q in range(NQT):
        base = q * KQ
        own = np.arange(base + 128 * a, base + 128 * a + P)
        others = np.setdiff1d(np.arange(base, base + KQ), own)
        perm.append(np.concatenate([others, own]))
    return np.concatenate(perm)


def _prep_in_maps(inputs, cfg):
    adt_np, pdt_np = _np_dt(cfg[0]), _np_dt(cfg[1])
    h = np.asarray(inputs["hidden_states"], dtype=np.float32)
    w_qkv = np.asarray(inputs["w_qkv"], np.float32)
    b_qkv = np.asarray(inputs["b_qkv"], np.float32)
    g1 = np.asarray(inputs["g1"], np.float32)
    be1 = np.asarray(inputs["be1"], np.float32)
    g2 = np.asarray(inputs["g2"], np.float32)
    be2 = np.asarray(inputs["be2"], np.float32)

    # fold LN1 gamma/beta into the qkv weights and biases
    w_qkv_f = w_qkv * g1[:, None]
    b_qkv_f = b_qkv + be1 @ w_qkv
    w_fc = np.asarray(inputs["w_fc"], np.float32)
    b_fc = np.asarray(inputs["b_fc"], np.float32)
    w_fc_f = w_fc * g2[:, None]
    b_fc_f = b_fc + be2 @ w_fc

    def chunk_w(w, p=P):  # [Din, N] -> [p, Din//p, N]
        return np.ascontiguousarray(w.reshape(-1, p, w.shape[1]).transpose(1, 0, 2))

    def pvec(v, p=P):  # [n*p] -> [p, n]
        return np.ascontiguousarray(v.reshape(-1, p).T)

    def mslice(a, nsl):  # [p, c, n] -> [n//nsl, p, c, nsl]
        p, c, n = a.shape
        return np.ascontiguousarray(
            a.reshape(p, c, n // nsl, nsl).transpose(2, 0, 1, 3)
        )

    wq = mslice(chunk_w(w_qkv_f[:, 0:D]), P)
    wk = mslice(chunk_w(w_qkv_f[:, D:2 * D]), P)
    wv = mslice(chunk_w(w_qkv_f[:, 2 * D:3 * D]), 512)

    # proj with head pairs (c, c+8) stacked on the 128 contraction rows
    w_proj = np.asarray(inputs["w_proj"], np.float32)
    w2 = w_proj.reshape(H, HD, D)
    wp_t = np.concatenate([w2[0:8], w2[8:16]], axis=1)  # [8, 128, 1024]
    wp = np.ascontiguousarray(
        wp_t.reshape(DC, P, DC, P).transpose(2, 1, 0, 3)
    )

    wfc = mslice(chunk_w(w_fc_f), P)
    wm = chunk_w(np.asarray(inputs["w_mlp"], np.float32))  # [128, 32, 1024]
    wmlp = np.ascontiguousarray(
        wm.reshape(P, 2, IC // 2, DC, P).transpose(1, 3, 0, 2, 4)
    )

    shared = {
        "w_q": wq.astype(adt_np), "w_k": wk.astype(adt_np),
        "w_v": wv.astype(adt_np), "w_projr": wp.astype(pdt_np),
        "w_fcr": wfc.astype(pdt_np), "w_mlpr": wmlp.astype(pdt_np),
        "bq": pvec(b_qkv_f[0:D]),
        "bk": pvec(b_qkv_f[D:2 * D]),
        "bv": pvec(b_qkv_f[2 * D:3 * D], p=HD),
        "bproj": pvec(np.asarray(inputs["b_proj"], np.float32)),
        "bfc": pvec(b_fc_f),
        "bmlp": pvec(np.asarray(inputs["b_mlp"], np.float32)),
    }
    in_maps = []
    for core in range(8):
        b, a = core // 4, core % 4
        perm = _perm_for_core(a)
        hrot = h[b, perm]
        # multiplicative 0/1 mask on exp'd scores for the diagonal query
        # tile: key position j=kt*128+p in the quarter vs query row r.
        pp = np.arange(P)
        dm = np.zeros((P, NKT, P), np.float32)
        for kt in range(NKT - 1):
            dm[:, kt, :] = ((kt * P + pp) < (P * a)).astype(np.float32)[:, None]
        dm[:, NKT - 1, :] = (pp[:, None] <= pp[None, :]).astype(np.float32)
        in_maps.append(
            dict(
                shared,
                hT=np.ascontiguousarray(hrot.T),
                dmask=dm.astype(adt_np),
            )
        )
    return in_maps


def _stitch(results):
    out = np.empty((2, S, D), dtype=np.float32)
    for core in range(8):
        b, a = core // 4, core % 4
        r = results[core]["outT"].T  # [512, D]: cols j*128+p -> row 512j+128a+p
        for j in range(NQT):
            out[b, j * KQ + P * a: j * KQ + P * a + P] = r[j * P:(j + 1) * P]
    return out


def run(inputs, cfg=("bf16", "bf16"), trace=False, trace_cores=None):
    nc = _get_nc(cfg)
    in_maps = _prep_in_maps(inputs, cfg)
    res = bass_utils.run_bass_kernel_spmd(
        nc, in_maps, core_ids=list(range(8)), trace=trace, trace_cores=trace_cores
    )
    return _stitch(res.results), res


def kernel(**inputs) -> np.ndarray:
    out, _ = run(inputs, cfg=("bf16", "bf16"))
    return out
